# revision 66
# baseline (speedup 1.0000x reference)
"""AGRU cell (antisymmetric GRU) forward on 8 TRN2 NeuronCores.

Data-parallel: batch 16384 is sharded 2048 rows/core; the six 1024x1024
weight matrices are replicated. No cross-core communication.

Everything on-device is computed in "hidden-major" (transposed) layout:
    zT = sigmoid(Wz @ xT + Uz @ hT + bz)        [H, B]
    rT = sigmoid(Wr @ xT + Ur @ hT + br)
    rhT = rT * hT
    dhT = tanh(Vh @ xT + A @ rhT + bh)
    outT = hT + eps * zT * dhT
so every matmul has the (pre-transposed, host-prepared) weight tile as the
stationary operand and xT/hT/rhT as the moving operand, and nothing ever
needs an on-device transpose.  The host transposes each core's [1024, 2048]
result back when assembling the full output.

Precision: the sigmoid-gate GEMMs (z, r) and the A@(r*h) term run in fp8
e4m3 with DoubleRow perf mode (2 contraction rows/cycle); inputs are
pre-scaled (x*16, W*256) and the scale is folded back in the activation
(scale=1/4096), which also applies the per-partition bias.  V_h@x runs in
bf16 (tanh passes its error through undamped, so it gets the accurate
path).  All accumulation is fp32 in PSUM; the residual h + eps*z*dh is
fp32 on the vector engine.  Measured rel err vs the fp32 reference: 9.1e-3.
"""

import sys

sys.path.insert(0, "/opt/trn_rl_repo")

import numpy as np
import ml_dtypes

from contextlib import ExitStack

import concourse.bass as bass
import concourse.mybir as mybir
from concourse import bacc, tile
from concourse.bass import ds, ts
from concourse.bass_utils import run_bass_kernel_spmd

BF16 = mybir.dt.bfloat16
FP8 = mybir.dt.float8e4
F32 = mybir.dt.float32
AFT = mybir.ActivationFunctionType
ALU = mybir.AluOpType
DR = mybir.MatmulPerfMode.DoubleRow

# fp8 pre-scaling for the sigmoid-gate GEMMs (z, r): data*16, weights*256,
# compensated by activation scale 1/(16*256).
SCALE_X = 16.0
SCALE_W = 256.0
INV_SCALE = 1.0 / (SCALE_X * SCALE_W)

N_CORES = 8
BATCH = 16384
B = BATCH // N_CORES  # per-core batch shard (2048)
H = 1024  # hidden == input size
KC = H // 128  # contraction chunks (8)
JT = H // 128  # output row tiles (8)
NB = 4  # moving-dim (batch) blocks per psum bank
NBS = B // NB  # 512 columns per matmul
GAMMA = 0.01

_nc_cache = {}


def _build(eps: float):
    """Build + compile the single-core Tile program (same graph on all cores)."""
    nc = bacc.Bacc("TRN2", target_bir_lowering=False, debug=False)

    xT_d = nc.dram_tensor("xT", [128, KC, B], BF16, kind="ExternalInput")
    hT_d = nc.dram_tensor("hT", [128, KC, B], BF16, kind="ExternalInput")
    # fp8 activations arrive batch-block-major so phase 1 can start on the
    # first 512-column block instead of waiting for the full stream.
    xT8_d = nc.dram_tensor("xT8", [NB, 128, KC, NBS], FP8, kind="ExternalInput")
    hT8_d = nc.dram_tensor("hT8", [NB, 128, KC, NBS], FP8, kind="ExternalInput")
    w_d = {
        name: nc.dram_tensor(name, [JT, 128, KC, 128], BF16, kind="ExternalInput")
        for name in ["vhT"]
    }
    w8_d = {
        name: nc.dram_tensor(name, [JT, 128, KC, 128], FP8, kind="ExternalInput")
        for name in ["wz8", "uz8", "wr8", "ur8", "at8"]
    }
    bias_d = nc.dram_tensor("biases", [128, 24], F32, kind="ExternalInput")
    out_d = nc.dram_tensor("out", [H, B], F32, kind="ExternalOutput")

    with tile.TileContext(nc) as tc, ExitStack() as ctx:
        singles = ctx.enter_context(tc.tile_pool(name="singles", bufs=1))
        wpool = ctx.enter_context(tc.tile_pool(name="wpool", bufs=6))
        rwpool = ctx.enter_context(tc.tile_pool(name="rwpool", bufs=16))
        psum = ctx.enter_context(tc.tile_pool(name="psum", bufs=8, space="PSUM"))
        actp = ctx.enter_context(tc.tile_pool(name="actp", bufs=10))
        tmpp = ctx.enter_context(tc.tile_pool(name="tmpp", bufs=4))
        outp = ctx.enter_context(tc.tile_pool(name="outp", bufs=2))

        xT = singles.tile([128, KC, B], BF16)
        hTb = singles.tile([128, KC, B], BF16)
        xT8 = singles.tile([128, KC, B], FP8)
        hT8 = singles.tile([128, KC, B], FP8)
        rhT8 = singles.tile([128, KC, B], FP8)
        bias_sb = singles.tile([128, 24], F32)

        def load_w(name, jt):
            # gpsimd (SWDGE): keeps weight blocks off the Sync HWDGE queue,
            # which is busy issuing the bulk x/h stream.
            fp8 = name in w8_d
            w = wpool.tile([128, KC, 128], FP8 if fp8 else BF16, tag="w")
            nc.gpsimd.dma_start(out=w[:], in_=(w8_d[name] if fp8 else w_d[name])[jt])
            return w

        def load_rw(name, jt):
            # phase-1 weights stay resident across all 4 batch-block passes
            w = rwpool.tile([128, KC, 128], FP8, tag="rw")
            nc.sync.dma_start(out=w[:], in_=w8_d[name][jt])
            return w

        # Issue order on the sync queue matches consumption order: jt0's
        # r-weights, the first x8/h8 batch block, then the remaining
        # r-weights interleaved with the hTb chunks (pass nb0 consumes
        # rw[jt] and hTb[:,jt,:] in lockstep, one pair every ~3.4us).
        # The jt0 weights and the first block are split into k-pair pieces so
        # the very first matmul only depends on ~160KB of DMA.
        wr0 = rwpool.tile([128, KC, 128], FP8, tag="rw", name="wr0")
        ur0 = rwpool.tile([128, KC, 128], FP8, tag="rw", name="ur0")
        nc.sync.dma_start(out=wr0[:, 0:2, :], in_=w8_d["wr8"][0][:, 0:2, :])
        nc.sync.dma_start(
            out=xT8[:, 0:2, ds(0, NBS)], in_=xT8_d[0][:, 0:2, :]
        )
        for kp in range(2, KC, 2):
            nc.sync.dma_start(
                out=wr0[:, kp : kp + 2, :], in_=w8_d["wr8"][0][:, kp : kp + 2, :]
            )
            nc.sync.dma_start(
                out=xT8[:, kp : kp + 2, ds(0, NBS)], in_=xT8_d[0][:, kp : kp + 2, :]
            )
        nc.sync.dma_start(out=ur0[:], in_=w8_d["ur8"][0])
        for kp in range(0, KC, 2):
            nc.sync.dma_start(
                out=hT8[:, kp : kp + 2, ds(0, NBS)], in_=hT8_d[0][:, kp : kp + 2, :]
            )
        rw = [(wr0, ur0)]
        nc.sync.dma_start(out=bias_sb[:], in_=bias_d[:])
        nc.sync.dma_start(out=hTb[:, 0, :], in_=hT_d[:, 0, :])
        for jt in range(1, JT):
            rw.append((load_rw("wr8", jt), load_rw("ur8", jt)))
            nc.sync.dma_start(out=hTb[:, jt, :], in_=hT_d[:, jt, :])
        # remaining x8/h8 blocks (needed from pass nb1 on) and bf16 x (phase 2)
        for nb in range(1, NB):
            nc.sync.dma_start(out=xT8[:, :, ds(nb * NBS, NBS)], in_=xT8_d[nb])
            nc.sync.dma_start(out=hT8[:, :, ds(nb * NBS, NBS)], in_=hT8_d[nb])
        for c in range(KC):
            nc.sync.dma_start(out=xT[:, c, :], in_=xT_d[:, c, :])

        def mm(psum_ap, w_ap, rhs_ap, start, stop, reload_w, perf_mode=None):
            # (LDWEIGHTS issue is left to legalization; measured to overlap
            # the matmul stream fully, so no manual dedupe/ordering needed.)
            return nc.tensor.matmul(
                psum_ap, w_ap, rhs_ap, start=start, stop=stop, perf_mode=perf_mode
            )

        def gemm_pair_fp8(psums, wA, rhsA, wB, rhsB):
            # fp8 DoubleRow, nb-outer: each PSUM bank completes (and can be
            # evicted by the ACT) while later banks are still accumulating.
            for nb in range(NB):
                for k in range(0, KC, 2):
                    mm(
                        psums[nb][:],
                        wA[:, k : k + 2, :],
                        rhsA[:, k : k + 2, ds(nb * NBS, NBS)],
                        start=(k == 0),
                        stop=False,
                        reload_w=False,
                        perf_mode=DR,
                    )
                for k in range(0, KC, 2):
                    mm(
                        psums[nb][:],
                        wB[:, k : k + 2, :],
                        rhsB[:, k : k + 2, ds(nb * NBS, NBS)],
                        start=False,
                        stop=(k == KC - 2),
                        reload_w=False,
                        perf_mode=DR,
                    )

        # ---- phase 1: r gate (hidden-major, fp8), rhT = sigmoid(...) * hT ----
        # Batch-block-outer, jt-inner: the first pass needs only the first
        # 512-column x8/h8 block plus the r weights, so the PE starts early
        # and never waits on the bulk DMA. The 8 jt groups of one pass
        # exactly fill the 8 PSUM banks.
        for nb in range(NB):
            for jt in range(JT):
                wr, ur = rw[jt]
                ps = psum.tile([128, NBS], F32, tag="ps", name=f"ps_r{jt}_{nb}")
                for k in range(0, KC, 2):
                    mm(
                        ps[:],
                        wr[:, k : k + 2, :],
                        xT8[:, k : k + 2, ds(nb * NBS, NBS)],
                        start=(k == 0),
                        stop=False,
                        reload_w=False,
                        perf_mode=DR,
                    )
                for k in range(0, KC, 2):
                    mm(
                        ps[:],
                        ur[:, k : k + 2, :],
                        hT8[:, k : k + 2, ds(nb * NBS, NBS)],
                        start=False,
                        stop=(k == KC - 2),
                        reload_w=False,
                        perf_mode=DR,
                    )
                rt = actp.tile([128, NBS], BF16, tag="act")
                nc.scalar.activation(
                    rt[:],
                    ps[:],
                    AFT.Sigmoid,
                    bias=bias_sb[:, 8 + jt : 9 + jt],
                    scale=INV_SCALE,
                )
                # rh in scaled fp8 for the DoubleRow A-matmul: (r*16)*h
                nc.vector.scalar_tensor_tensor(
                    rhT8[:, jt, ds(nb * NBS, NBS)],
                    rt[:],
                    SCALE_X,
                    hTb[:, jt, ds(nb * NBS, NBS)],
                    op0=ALU.mult,
                    op1=ALU.mult,
                )

        # ---- phase 2: z gate (fp8) + delta_h (bf16) + residual, per jt ----
        for jt in range(JT):
            wz = load_w("wz8", jt)
            uz = load_w("uz8", jt)
            vh = load_w("vhT", jt)
            at = load_w("at8", jt)
            psz = [
                psum.tile([128, NBS], F32, tag="ps", name=f"ps_z{jt}_{i}")
                for i in range(NB)
            ]
            gemm_pair_fp8(psz, wz, xT8, uz, hT8)
            # dh-pre: V_h@x in bf16 (V_h host-scaled by 4096) and A@(r*h) in
            # fp8 DoubleRow (scales 16*256) — separate PSUM groups (mixing
            # perf modes in one accumulation group faults the device), summed
            # on the DVE.
            psv = [
                psum.tile([128, NBS], F32, tag="ps", name=f"ps_v{jt}_{i}")
                for i in range(NB)
            ]
            vhs = []
            for nb in range(NB):
                for k in range(KC):
                    mm(
                        psv[nb][:],
                        vh[:, k, :],
                        xT[:, k, ds(nb * NBS, NBS)],
                        start=(k == 0),
                        stop=(k == KC - 1),
                        reload_w=False,
                    )
                t = tmpp.tile([128, NBS], F32, tag="vhs", name=f"vhs{jt}_{nb}")
                nc.vector.tensor_copy(t[:], psv[nb][:])
                vhs.append(t)
            psd = [
                psum.tile([128, NBS], F32, tag="ps", name=f"ps_d{jt}_{i}")
                for i in range(NB)
            ]
            for nb in range(NB):
                for k in range(0, KC, 2):
                    mm(
                        psd[nb][:],
                        at[:, k : k + 2, :],
                        rhT8[:, k : k + 2, ds(nb * NBS, NBS)],
                        start=(k == 0),
                        stop=(k == KC - 2),
                        reload_w=False,
                        perf_mode=DR,
                    )
            ot = outp.tile([128, B], F32, tag="out")
            for nb in range(NB):
                zt = actp.tile([128, NBS], BF16, tag="act")
                nc.scalar.activation(
                    zt[:],
                    psz[nb][:],
                    AFT.Sigmoid,
                    bias=bias_sb[:, jt : jt + 1],
                    scale=INV_SCALE,
                )
                dsum = tmpp.tile([128, NBS], F32, tag="dsum")
                nc.vector.tensor_add(dsum[:], psd[nb][:], vhs[nb][:])
                dt_ = actp.tile([128, NBS], BF16, tag="act")
                nc.scalar.activation(
                    dt_[:],
                    dsum[:],
                    AFT.Tanh,
                    bias=bias_sb[:, 16 + jt : 17 + jt],
                    scale=INV_SCALE,
                )
                zdh = tmpp.tile([128, NBS], F32, tag="zdh")
                nc.vector.tensor_mul(zdh[:], zt[:], dt_[:])
                # out = (z*dh) * eps + h
                nc.vector.scalar_tensor_tensor(
                    ot[:, ds(nb * NBS, NBS)],
                    zdh[:],
                    float(eps),
                    hTb[:, jt, ds(nb * NBS, NBS)],
                    op0=ALU.mult,
                    op1=ALU.add,
                )
                nc.sync.dma_start(
                    out=out_d[ts(jt, 128), ds(nb * NBS, NBS)],
                    in_=ot[:, ds(nb * NBS, NBS)],
                )

    nc.compile()
    return nc


def _get_nc(eps: float):
    key = float(eps)
    if key not in _nc_cache:
        _nc_cache[key] = _build(key)
    return _nc_cache[key]


def _block_weight(wT, dtype, scale=1.0):
    # [1024, 1024] (contraction-major) -> [jt, p, c, j] st. blk[jt,p,c,j] = wT[c*128+p, jt*128+j]
    blk = wT.reshape(KC, 128, JT, 128).transpose(2, 1, 0, 3)
    if scale != 1.0:
        blk = blk * scale
    return np.ascontiguousarray(blk).astype(dtype)


def _block_data(m):
    # per-core [B, 1024] -> [p, c, b] st. blk[p,c,b] = m[b, c*128+p]
    return np.ascontiguousarray(m.T.reshape(KC, 128, B).transpose(1, 0, 2))


def _block_data_nb(m):
    # per-core [B, 1024] -> [nb, p, c, nbs] batch-block-major
    blk = m.T.reshape(KC, 128, NB, NBS).transpose(2, 1, 0, 3)
    return np.ascontiguousarray(blk)


def _prep_in_maps(x, h_prev, W_z, b_z, U_z, W_r, b_r, U_r, V_h, b_h, W_h):
    BF = ml_dtypes.bfloat16
    F8 = ml_dtypes.float8_e4m3
    x16 = np.asarray(x, np.float32).astype(BF)
    h16 = np.asarray(h_prev, np.float32).astype(BF)
    x8 = (np.asarray(x, np.float32) * SCALE_X).astype(F8)
    h8 = (np.asarray(h_prev, np.float32) * SCALE_X).astype(F8)

    A = W_h - W_h.T - GAMMA * np.eye(H, dtype=np.float32)
    shared = {
        "wz8": _block_weight(W_z.T, F8, SCALE_W),
        "uz8": _block_weight(U_z.T, F8, SCALE_W),
        "wr8": _block_weight(W_r.T, F8, SCALE_W),
        "ur8": _block_weight(U_r.T, F8, SCALE_W),
        "at8": _block_weight(A.T, F8, SCALE_W),
        "vhT": _block_weight(V_h.T, BF, SCALE_X * SCALE_W),
        "biases": np.ascontiguousarray(
            np.concatenate(
                [
                    b_z.reshape(JT, 128).T,
                    b_r.reshape(JT, 128).T,
                    b_h.reshape(JT, 128).T,
                ],
                axis=1,
            ).astype(np.float32)
        ),
    }
    in_maps = []
    for c in range(N_CORES):
        sl = slice(c * B, (c + 1) * B)
        in_maps.append(
            {
                "xT": _block_data(x16[sl]),
                "hT": _block_data(h16[sl]),
                "xT8": _block_data_nb(x8[sl]),
                "hT8": _block_data_nb(h8[sl]),
                **shared,
            }
        )
    return in_maps


def run(inputs, trace=False):
    """Returns (full_output [16384,1024] f32, BassKernelResults)."""
    np_in = {k: np.asarray(v, np.float32) for k, v in inputs.items()}
    eps = float(np_in.pop("epsilon"))
    in_maps = _prep_in_maps(**np_in)
    nc = _get_nc(eps)
    res = run_bass_kernel_spmd(
        nc, in_maps, core_ids=list(range(N_CORES)), trace=trace
    )
    out = np.empty((BATCH, H), np.float32)
    for c in range(N_CORES):
        out[c * B : (c + 1) * B, :] = res.results[c]["out"].T
    return out, res


def kernel(**inputs) -> np.ndarray:
    out, _ = run(inputs, trace=False)
    return out


# revision 67
# speedup vs baseline: 1.0129x; 1.0129x over previous
"""AGRU cell (antisymmetric GRU) forward on 8 TRN2 NeuronCores.

Data-parallel: batch 16384 is sharded 2048 rows/core; the six 1024x1024
weight matrices are replicated. No cross-core communication.

Everything on-device is computed in "hidden-major" (transposed) layout:
    zT = sigmoid(Wz @ xT + Uz @ hT + bz)        [H, B]
    rT = sigmoid(Wr @ xT + Ur @ hT + br)
    rhT = rT * hT
    dhT = tanh(Vh @ xT + A @ rhT + bh)
    outT = hT + eps * zT * dhT
so every matmul has the (pre-transposed, host-prepared) weight tile as the
stationary operand and xT/hT/rhT as the moving operand, and nothing ever
needs an on-device transpose.  The host transposes each core's [1024, 2048]
result back when assembling the full output.

Precision: the sigmoid-gate GEMMs (z, r) and the A@(r*h) term run in fp8
e4m3 with DoubleRow perf mode (2 contraction rows/cycle); inputs are
pre-scaled (x*16, W*256) and the scale is folded back in the activation
(scale=1/4096), which also applies the per-partition bias.  V_h@x runs in
bf16 (tanh passes its error through undamped, so it gets the accurate
path).  All accumulation is fp32 in PSUM; the residual h + eps*z*dh is
fp32 on the vector engine.  Measured rel err vs the fp32 reference: 9.1e-3.
"""

import sys

sys.path.insert(0, "/opt/trn_rl_repo")

import numpy as np
import ml_dtypes

from contextlib import ExitStack

import concourse.bass as bass
import concourse.mybir as mybir
from concourse import bacc, tile
from concourse.bass import ds, ts
from concourse.bass_utils import run_bass_kernel_spmd
from concourse.tile_rust import add_dep_helper

BF16 = mybir.dt.bfloat16
FP8 = mybir.dt.float8e4
F32 = mybir.dt.float32
AFT = mybir.ActivationFunctionType
ALU = mybir.AluOpType
DR = mybir.MatmulPerfMode.DoubleRow

# fp8 pre-scaling for the sigmoid-gate GEMMs (z, r): data*16, weights*256,
# compensated by activation scale 1/(16*256).
SCALE_X = 16.0
SCALE_W = 256.0
INV_SCALE = 1.0 / (SCALE_X * SCALE_W)

N_CORES = 8
BATCH = 16384
B = BATCH // N_CORES  # per-core batch shard (2048)
H = 1024  # hidden == input size
KC = H // 128  # contraction chunks (8)
JT = H // 128  # output row tiles (8)
NB = 4  # moving-dim (batch) blocks per psum bank
NBS = B // NB  # 512 columns per matmul
GAMMA = 0.01

_nc_cache = {}


def _build(eps: float):
    """Build + compile the single-core Tile program (same graph on all cores)."""
    nc = bacc.Bacc("TRN2", target_bir_lowering=False, debug=False)

    xT_d = nc.dram_tensor("xT", [128, KC, B], BF16, kind="ExternalInput")
    hT_d = nc.dram_tensor("hT", [128, KC, B], BF16, kind="ExternalInput")
    # fp8 activations arrive batch-block-major so phase 1 can start on the
    # first 512-column block instead of waiting for the full stream.
    xT8_d = nc.dram_tensor("xT8", [NB, 128, KC, NBS], FP8, kind="ExternalInput")
    hT8_d = nc.dram_tensor("hT8", [NB, 128, KC, NBS], FP8, kind="ExternalInput")
    w_d = {
        name: nc.dram_tensor(name, [JT, 128, KC, 128], BF16, kind="ExternalInput")
        for name in ["vhT"]
    }
    w8_d = {
        name: nc.dram_tensor(name, [JT, 128, KC, 128], FP8, kind="ExternalInput")
        for name in ["wz8", "uz8", "wr8", "ur8", "at8"]
    }
    bias_d = nc.dram_tensor("biases", [128, 24], F32, kind="ExternalInput")
    out_d = nc.dram_tensor("out", [H, B], F32, kind="ExternalOutput")

    with tile.TileContext(nc) as tc, ExitStack() as ctx:
        singles = ctx.enter_context(tc.tile_pool(name="singles", bufs=1))
        wpool = ctx.enter_context(tc.tile_pool(name="wpool", bufs=6))
        rwpool = ctx.enter_context(tc.tile_pool(name="rwpool", bufs=16))
        psum = ctx.enter_context(tc.tile_pool(name="psum", bufs=8, space="PSUM"))
        actp = ctx.enter_context(tc.tile_pool(name="actp", bufs=10))
        tmpp = ctx.enter_context(tc.tile_pool(name="tmpp", bufs=4))
        outp = ctx.enter_context(tc.tile_pool(name="outp", bufs=2))

        xT = singles.tile([128, KC, B], BF16)
        hTb = singles.tile([128, KC, B], BF16)
        xT8 = singles.tile([128, KC, B], FP8)
        hT8 = singles.tile([128, KC, B], FP8)
        rhT8 = singles.tile([128, KC, B], FP8)
        bias_sb = singles.tile([128, 24], F32)

        def load_w(name, jt):
            # gpsimd (SWDGE): keeps weight blocks off the Sync HWDGE queue,
            # which is busy issuing the bulk x/h stream.
            fp8 = name in w8_d
            w = wpool.tile([128, KC, 128], FP8 if fp8 else BF16, tag="w")
            nc.gpsimd.dma_start(out=w[:], in_=(w8_d[name] if fp8 else w_d[name])[jt])
            return w

        def load_rw(name, jt):
            # phase-1 weights stay resident across all 4 batch-block passes
            w = rwpool.tile([128, KC, 128], FP8, tag="rw")
            nc.sync.dma_start(out=w[:], in_=w8_d[name][jt])
            return w

        # Issue order on the sync queue matches consumption order: jt0's
        # r-weights, the first x8/h8 batch block, then the remaining
        # r-weights interleaved with the hTb chunks (pass nb0 consumes
        # rw[jt] and hTb[:,jt,:] in lockstep, one pair every ~3.4us).
        rw = [(load_rw("wr8", 0), load_rw("ur8", 0))]
        # first batch block split by contraction chunk-pairs, x before h,
        # matching the k-sweep consumption order of the first matmul group
        for kp in range(0, KC, 2):
            nc.sync.dma_start(
                out=xT8[:, kp : kp + 2, ds(0, NBS)], in_=xT8_d[0][:, kp : kp + 2, :]
            )
        for kp in range(0, KC, 2):
            nc.sync.dma_start(
                out=hT8[:, kp : kp + 2, ds(0, NBS)], in_=hT8_d[0][:, kp : kp + 2, :]
            )
        nc.sync.dma_start(out=bias_sb[:], in_=bias_d[:])
        nc.sync.dma_start(out=hTb[:, 0, :], in_=hT_d[:, 0, :])
        for jt in range(1, JT):
            rw.append((load_rw("wr8", jt), load_rw("ur8", jt)))
            nc.sync.dma_start(out=hTb[:, jt, :], in_=hT_d[:, jt, :])
        # remaining x8/h8 blocks (needed from pass nb1 on) and bf16 x (phase 2)
        for nb in range(1, NB):
            nc.sync.dma_start(out=xT8[:, :, ds(nb * NBS, NBS)], in_=xT8_d[nb])
            nc.sync.dma_start(out=hT8[:, :, ds(nb * NBS, NBS)], in_=hT8_d[nb])
        for c in range(KC):
            nc.sync.dma_start(out=xT[:, c, :], in_=xT_d[:, c, :])

        # All PE matmuls are chained in program order (ordering-only deps) so
        # that groups of matmuls sharing a stationary operand stay contiguous:
        # followers in each group skip their LDWEIGHTS (ldweights=False) and
        # reuse the weights already in the array.
        prev_mm = [None]

        USE_LDW_DEDUPE = False

        def mm(psum_ap, w_ap, rhs_ap, start, stop, reload_w, perf_mode=None):
            bi = nc.tensor.matmul(
                psum_ap, w_ap, rhs_ap, start=start, stop=stop, perf_mode=perf_mode
            )
            if USE_LDW_DEDUPE:
                if not reload_w:
                    bi.ins.ldweights = False
                if prev_mm[0] is not None:
                    add_dep_helper(bi.ins, prev_mm[0], False, "pe-order")
                prev_mm[0] = bi.ins
            return bi

        def gemm_pair(psums, wA, rhsA, wB, rhsB):
            # psums[nb] += wA[:,k,:].T @ rhsA[:,k,nb] summed over k, then wB/rhsB
            for k in range(KC):
                for nb in range(NB):
                    mm(
                        psums[nb][:],
                        wA[:, k, :],
                        rhsA[:, k, ds(nb * NBS, NBS)],
                        start=(k == 0),
                        stop=False,
                        reload_w=(nb == 0),
                    )
            for k in range(KC):
                for nb in range(NB):
                    mm(
                        psums[nb][:],
                        wB[:, k, :],
                        rhsB[:, k, ds(nb * NBS, NBS)],
                        start=False,
                        stop=(k == KC - 1),
                        reload_w=(nb == 0),
                    )

        def gemm_pair_fp8(psums, wA, rhsA, wB, rhsB):
            # fp8 DoubleRow, nb-outer: each PSUM bank completes (and is
            # evicted by the ACT) while later banks still accumulate.
            for nb in range(NB):
                for k in range(0, KC, 2):
                    mm(
                        psums[nb][:],
                        wA[:, k : k + 2, :],
                        rhsA[:, k : k + 2, ds(nb * NBS, NBS)],
                        start=(k == 0),
                        stop=False,
                        reload_w=False,
                        perf_mode=DR,
                    )
                for k in range(0, KC, 2):
                    mm(
                        psums[nb][:],
                        wB[:, k : k + 2, :],
                        rhsB[:, k : k + 2, ds(nb * NBS, NBS)],
                        start=False,
                        stop=(k == KC - 2),
                        reload_w=False,
                        perf_mode=DR,
                    )

        # ---- phase 1: r gate (hidden-major, fp8), rhT = sigmoid(...) * hT ----
        # Batch-block-outer, jt-inner: the first pass needs only the first
        # 512-column x8/h8 block plus the r weights, so the PE starts early
        # and never waits on the bulk DMA. The 8 jt groups of one pass
        # exactly fill the 8 PSUM banks.
        for nb in range(NB):
            for jt in range(JT):
                wr, ur = rw[jt]
                ps = psum.tile([128, NBS], F32, tag="ps", name=f"ps_r{jt}_{nb}")
                for k in range(0, KC, 2):
                    mm(
                        ps[:],
                        wr[:, k : k + 2, :],
                        xT8[:, k : k + 2, ds(nb * NBS, NBS)],
                        start=(k == 0),
                        stop=False,
                        reload_w=False,
                        perf_mode=DR,
                    )
                for k in range(0, KC, 2):
                    mm(
                        ps[:],
                        ur[:, k : k + 2, :],
                        hT8[:, k : k + 2, ds(nb * NBS, NBS)],
                        start=False,
                        stop=(k == KC - 2),
                        reload_w=False,
                        perf_mode=DR,
                    )
                rt = actp.tile([128, NBS], BF16, tag="act")
                nc.scalar.activation(
                    rt[:],
                    ps[:],
                    AFT.Sigmoid,
                    bias=bias_sb[:, 8 + jt : 9 + jt],
                    scale=INV_SCALE,
                )
                # rh in scaled fp8 for the DoubleRow A-matmul: (r*16)*h
                nc.vector.scalar_tensor_tensor(
                    rhT8[:, jt, ds(nb * NBS, NBS)],
                    rt[:],
                    SCALE_X,
                    hTb[:, jt, ds(nb * NBS, NBS)],
                    op0=ALU.mult,
                    op1=ALU.mult,
                )

        # ---- phase 2: z gate (fp8) + delta_h (bf16) + residual, per jt ----
        for jt in range(JT):
            wz = load_w("wz8", jt)
            uz = load_w("uz8", jt)
            vh = load_w("vhT", jt)
            at = load_w("at8", jt)
            psz = [
                psum.tile([128, NBS], F32, tag="ps", name=f"ps_z{jt}_{i}")
                for i in range(NB)
            ]
            gemm_pair_fp8(psz, wz, xT8, uz, hT8)
            # dh-pre: V_h@x in bf16 (V_h host-scaled by 4096) and A@(r*h) in
            # fp8 DoubleRow (scales 16*256) — separate PSUM groups (mixing
            # perf modes in one accumulation group faults the device), summed
            # on the DVE.
            psv = [
                psum.tile([128, NBS], F32, tag="ps", name=f"ps_v{jt}_{i}")
                for i in range(NB)
            ]
            vhs = []
            for nb in range(NB):
                for k in range(KC):
                    mm(
                        psv[nb][:],
                        vh[:, k, :],
                        xT[:, k, ds(nb * NBS, NBS)],
                        start=(k == 0),
                        stop=(k == KC - 1),
                        reload_w=False,
                    )
                t = tmpp.tile([128, NBS], F32, tag="vhs", name=f"vhs{jt}_{nb}")
                nc.vector.tensor_copy(t[:], psv[nb][:])
                vhs.append(t)
            psd = [
                psum.tile([128, NBS], F32, tag="ps", name=f"ps_d{jt}_{i}")
                for i in range(NB)
            ]
            for nb in range(NB):
                for k in range(0, KC, 2):
                    mm(
                        psd[nb][:],
                        at[:, k : k + 2, :],
                        rhT8[:, k : k + 2, ds(nb * NBS, NBS)],
                        start=(k == 0),
                        stop=(k == KC - 2),
                        reload_w=False,
                        perf_mode=DR,
                    )
            ot = outp.tile([128, B], F32, tag="out")
            for nb in range(NB):
                zt = actp.tile([128, NBS], BF16, tag="act")
                nc.scalar.activation(
                    zt[:],
                    psz[nb][:],
                    AFT.Sigmoid,
                    bias=bias_sb[:, jt : jt + 1],
                    scale=INV_SCALE,
                )
                dsum = tmpp.tile([128, NBS], F32, tag="dsum")
                nc.vector.tensor_add(dsum[:], psd[nb][:], vhs[nb][:])
                dt_ = actp.tile([128, NBS], BF16, tag="act")
                nc.scalar.activation(
                    dt_[:],
                    dsum[:],
                    AFT.Tanh,
                    bias=bias_sb[:, 16 + jt : 17 + jt],
                    scale=INV_SCALE,
                )
                zdh = tmpp.tile([128, NBS], F32, tag="zdh")
                nc.vector.tensor_mul(zdh[:], zt[:], dt_[:])
                # out = (z*dh) * eps + h
                nc.vector.scalar_tensor_tensor(
                    ot[:, ds(nb * NBS, NBS)],
                    zdh[:],
                    float(eps),
                    hTb[:, jt, ds(nb * NBS, NBS)],
                    op0=ALU.mult,
                    op1=ALU.add,
                )
                nc.sync.dma_start(
                    out=out_d[ts(jt, 128), ds(nb * NBS, NBS)],
                    in_=ot[:, ds(nb * NBS, NBS)],
                )

    nc.compile()
    return nc


def _dedupe_ldweights(nc):
    """Drop back-to-back InstLdweights with identical weight APs.

    Tile legalization splits every bf16 matmul into LDWEIGHTS+MATMUL even when
    consecutive matmuls share the stationary operand. The PE executes its
    stream in order, so a repeated load of the same weights is pure overhead
    (~128 cycles per 512-cycle matmul). Only drops loads that carry no
    semaphore waits/updates; the explicit pe-order dep chain built in _build
    guarantees groups sharing weights are contiguous in the stream.
    """
    removed = 0
    for blk in nc.m.functions[0].blocks:
        new = []
        last_key = None
        for i in blk.instructions:
            if i.engine == mybir.EngineType.PE:
                if isinstance(i, mybir.InstLdweights):
                    si = i.sync_info
                    clean = si is None or (not si.on_wait and not si.on_update)
                    key = str(i.ins[0])
                    if clean and key == last_key:
                        removed += 1
                        continue
                    last_key = key
                elif not isinstance(i, mybir.InstMatmult):
                    last_key = None
            new.append(i)
        blk.instructions[:] = new
    return removed


def _get_nc(eps: float):
    key = float(eps)
    if key not in _nc_cache:
        _nc_cache[key] = _build(key)
    return _nc_cache[key]


def _block_weight(wT, dtype, scale=1.0):
    # [1024, 1024] (contraction-major) -> [jt, p, c, j] st. blk[jt,p,c,j] = wT[c*128+p, jt*128+j]
    blk = wT.reshape(KC, 128, JT, 128).transpose(2, 1, 0, 3)
    if scale != 1.0:
        blk = blk * scale
    return np.ascontiguousarray(blk).astype(dtype)


def _block_data(m):
    # per-core [B, 1024] -> [p, c, b] st. blk[p,c,b] = m[b, c*128+p]
    return np.ascontiguousarray(m.T.reshape(KC, 128, B).transpose(1, 0, 2))


def _block_data_nb(m):
    # per-core [B, 1024] -> [nb, p, c, nbs] batch-block-major
    blk = m.T.reshape(KC, 128, NB, NBS).transpose(2, 1, 0, 3)
    return np.ascontiguousarray(blk)


def _prep_in_maps(x, h_prev, W_z, b_z, U_z, W_r, b_r, U_r, V_h, b_h, W_h):
    BF = ml_dtypes.bfloat16
    F8 = ml_dtypes.float8_e4m3
    x16 = np.asarray(x, np.float32).astype(BF)
    h16 = np.asarray(h_prev, np.float32).astype(BF)
    x8 = (np.asarray(x, np.float32) * SCALE_X).astype(F8)
    h8 = (np.asarray(h_prev, np.float32) * SCALE_X).astype(F8)

    A = W_h - W_h.T - GAMMA * np.eye(H, dtype=np.float32)
    shared = {
        "wz8": _block_weight(W_z.T, F8, SCALE_W),
        "uz8": _block_weight(U_z.T, F8, SCALE_W),
        "wr8": _block_weight(W_r.T, F8, SCALE_W),
        "ur8": _block_weight(U_r.T, F8, SCALE_W),
        "at8": _block_weight(A.T, F8, SCALE_W),
        "vhT": _block_weight(V_h.T, BF, SCALE_X * SCALE_W),
        "biases": np.ascontiguousarray(
            np.concatenate(
                [
                    b_z.reshape(JT, 128).T,
                    b_r.reshape(JT, 128).T,
                    b_h.reshape(JT, 128).T,
                ],
                axis=1,
            ).astype(np.float32)
        ),
    }
    in_maps = []
    for c in range(N_CORES):
        sl = slice(c * B, (c + 1) * B)
        in_maps.append(
            {
                "xT": _block_data(x16[sl]),
                "hT": _block_data(h16[sl]),
                "xT8": _block_data_nb(x8[sl]),
                "hT8": _block_data_nb(h8[sl]),
                **shared,
            }
        )
    return in_maps


def run(inputs, trace=False):
    """Returns (full_output [16384,1024] f32, BassKernelResults)."""
    np_in = {k: np.asarray(v, np.float32) for k, v in inputs.items()}
    eps = float(np_in.pop("epsilon"))
    in_maps = _prep_in_maps(**np_in)
    nc = _get_nc(eps)
    res = run_bass_kernel_spmd(
        nc, in_maps, core_ids=list(range(N_CORES)), trace=trace
    )
    out = np.empty((BATCH, H), np.float32)
    for c in range(N_CORES):
        out[c * B : (c + 1) * B, :] = res.results[c]["out"].T
    return out, res


def kernel(**inputs) -> np.ndarray:
    out, _ = run(inputs, trace=False)
    return out


# revision 68
# speedup vs baseline: 1.0248x; 1.0118x over previous
"""AGRU cell (antisymmetric GRU) forward on 8 TRN2 NeuronCores.

Data-parallel: batch 16384 is sharded 2048 rows/core; the six 1024x1024
weight matrices are replicated. No cross-core communication.

Everything on-device is computed in "hidden-major" (transposed) layout:
    zT = sigmoid(Wz @ xT + Uz @ hT + bz)        [H, B]
    rT = sigmoid(Wr @ xT + Ur @ hT + br)
    rhT = rT * hT
    dhT = tanh(Vh @ xT + A @ rhT + bh)
    outT = hT + eps * zT * dhT
so every matmul has the (pre-transposed, host-prepared) weight tile as the
stationary operand and xT/hT/rhT as the moving operand, and nothing ever
needs an on-device transpose.  The host transposes each core's [1024, 2048]
result back when assembling the full output.

Precision: the sigmoid-gate GEMMs (z, r) and the A@(r*h) term run in fp8
e4m3 with DoubleRow perf mode (2 contraction rows/cycle); inputs are
pre-scaled (x*16, W*256) and the scale is folded back in the activation
(scale=1/4096), which also applies the per-partition bias.  V_h@x runs in
bf16 (tanh passes its error through undamped, so it gets the accurate
path).  All accumulation is fp32 in PSUM; the residual h + eps*z*dh is
fp32 on the vector engine.  Measured rel err vs the fp32 reference: 9.1e-3.
"""

import sys

sys.path.insert(0, "/opt/trn_rl_repo")

import numpy as np
import ml_dtypes

from contextlib import ExitStack

import concourse.bass as bass
import concourse.mybir as mybir
from concourse import bacc, tile
from concourse.bass import ds, ts
from concourse.bass_utils import run_bass_kernel_spmd

BF16 = mybir.dt.bfloat16
FP8 = mybir.dt.float8e4
F32 = mybir.dt.float32
AFT = mybir.ActivationFunctionType
ALU = mybir.AluOpType
DR = mybir.MatmulPerfMode.DoubleRow

# fp8 pre-scaling for the sigmoid-gate GEMMs (z, r): data*16, weights*256,
# compensated by activation scale 1/(16*256).
SCALE_X = 16.0
SCALE_W = 256.0
INV_SCALE = 1.0 / (SCALE_X * SCALE_W)

N_CORES = 8
BATCH = 16384
B = BATCH // N_CORES  # per-core batch shard (2048)
H = 1024  # hidden == input size
KC = H // 128  # contraction chunks (8)
JT = H // 128  # output row tiles (8)
NB = 4  # moving-dim (batch) blocks per psum bank
NBS = B // NB  # 512 columns per matmul
GAMMA = 0.01

_nc_cache = {}


def _build(eps: float):
    """Build + compile the single-core Tile program (same graph on all cores)."""
    nc = bacc.Bacc("TRN2", target_bir_lowering=False, debug=False)

    xT_d = nc.dram_tensor("xT", [128, KC, B], BF16, kind="ExternalInput")
    hT_d = nc.dram_tensor("hT", [128, KC, B], BF16, kind="ExternalInput")
    # fp8 activations arrive batch-block-major so phase 1 can start on the
    # first 512-column block instead of waiting for the full stream.
    xT8_d = nc.dram_tensor("xT8", [NB, 128, KC, NBS], FP8, kind="ExternalInput")
    hT8_d = nc.dram_tensor("hT8", [NB, 128, KC, NBS], FP8, kind="ExternalInput")
    w_d = {
        name: nc.dram_tensor(name, [JT, 128, KC, 128], BF16, kind="ExternalInput")
        for name in ["vhT"]
    }
    w8_d = {
        name: nc.dram_tensor(name, [JT, 128, KC, 128], FP8, kind="ExternalInput")
        for name in ["wz8", "uz8", "wr8", "ur8", "at8"]
    }
    bias_d = nc.dram_tensor("biases", [128, 24], F32, kind="ExternalInput")
    out_d = nc.dram_tensor("out", [H, B], F32, kind="ExternalOutput")

    with tile.TileContext(nc) as tc, ExitStack() as ctx:
        singles = ctx.enter_context(tc.tile_pool(name="singles", bufs=1))
        wpool = ctx.enter_context(tc.tile_pool(name="wpool", bufs=6))
        rwpool = ctx.enter_context(tc.tile_pool(name="rwpool", bufs=16))
        psum = ctx.enter_context(tc.tile_pool(name="psum", bufs=8, space="PSUM"))
        actp = ctx.enter_context(tc.tile_pool(name="actp", bufs=10))
        tmpp = ctx.enter_context(tc.tile_pool(name="tmpp", bufs=4))
        outp = ctx.enter_context(tc.tile_pool(name="outp", bufs=2))

        xT = singles.tile([128, KC, B], BF16)
        hTb = singles.tile([128, KC, B], BF16)
        xT8 = singles.tile([128, KC, B], FP8)
        hT8 = singles.tile([128, KC, B], FP8)
        rhT8 = singles.tile([128, KC, B], FP8)
        bias_sb = singles.tile([128, 24], F32)

        def load_w(name, jt):
            # gpsimd (SWDGE): keeps weight blocks off the Sync HWDGE queue,
            # which is busy issuing the bulk x/h stream.
            fp8 = name in w8_d
            w = wpool.tile([128, KC, 128], FP8 if fp8 else BF16, tag="w")
            nc.gpsimd.dma_start(out=w[:], in_=(w8_d[name] if fp8 else w_d[name])[jt])
            return w

        def load_rw(name, jt):
            # phase-1 weights stay resident across all 4 batch-block passes
            w = rwpool.tile([128, KC, 128], FP8, tag="rw")
            nc.sync.dma_start(out=w[:], in_=w8_d[name][jt])
            return w

        # Issue order on the sync queue matches consumption order: jt0's
        # r-weights, the first x8/h8 batch block, then the remaining
        # r-weights interleaved with the hTb chunks (pass nb0 consumes
        # rw[jt] and hTb[:,jt,:] in lockstep, one pair every ~3.4us).
        rw = [(load_rw("wr8", 0), load_rw("ur8", 0))]
        # first batch block split by contraction chunk-pairs, x before h,
        # matching the k-sweep consumption order of the first matmul group
        for kp in range(0, KC, 2):
            nc.sync.dma_start(
                out=xT8[:, kp : kp + 2, ds(0, NBS)], in_=xT8_d[0][:, kp : kp + 2, :]
            )
        for kp in range(0, KC, 2):
            nc.sync.dma_start(
                out=hT8[:, kp : kp + 2, ds(0, NBS)], in_=hT8_d[0][:, kp : kp + 2, :]
            )
        nc.sync.dma_start(out=bias_sb[:], in_=bias_d[:])
        nc.sync.dma_start(out=hTb[:, 0, :], in_=hT_d[:, 0, :])
        for jt in range(1, JT):
            rw.append((load_rw("wr8", jt), load_rw("ur8", jt)))
            nc.sync.dma_start(out=hTb[:, jt, :], in_=hT_d[:, jt, :])
        # remaining x8/h8 blocks (needed from pass nb1 on) and bf16 x (phase 2)
        for nb in range(1, NB):
            nc.sync.dma_start(out=xT8[:, :, ds(nb * NBS, NBS)], in_=xT8_d[nb])
            nc.sync.dma_start(out=hT8[:, :, ds(nb * NBS, NBS)], in_=hT8_d[nb])
        for c in range(KC):
            nc.sync.dma_start(out=xT[:, c, :], in_=xT_d[:, c, :])

        def mm(psum_ap, w_ap, rhs_ap, start, stop, reload_w, perf_mode=None):
            # (LDWEIGHTS issue is left to legalization; measured to overlap
            # the matmul stream fully, so no manual dedupe/ordering needed.)
            return nc.tensor.matmul(
                psum_ap, w_ap, rhs_ap, start=start, stop=stop, perf_mode=perf_mode
            )

        def gemm_pair_fp8(psums, wA, rhsA, wB, rhsB):
            # fp8 DoubleRow, nb-outer: each PSUM bank completes (and is
            # evicted by the ACT) while later banks still accumulate.
            for nb in range(NB):
                for k in range(0, KC, 2):
                    mm(
                        psums[nb][:],
                        wA[:, k : k + 2, :],
                        rhsA[:, k : k + 2, ds(nb * NBS, NBS)],
                        start=(k == 0),
                        stop=False,
                        reload_w=False,
                        perf_mode=DR,
                    )
                for k in range(0, KC, 2):
                    mm(
                        psums[nb][:],
                        wB[:, k : k + 2, :],
                        rhsB[:, k : k + 2, ds(nb * NBS, NBS)],
                        start=False,
                        stop=(k == KC - 2),
                        reload_w=False,
                        perf_mode=DR,
                    )

        # ---- phase 1: r gate (hidden-major, fp8), rhT = sigmoid(...) * hT ----
        # Batch-block-outer, jt-inner: the first pass needs only the first
        # 512-column x8/h8 block plus the r weights, so the PE starts early
        # and never waits on the bulk DMA. The 8 jt groups of one pass
        # exactly fill the 8 PSUM banks.
        for nb in range(NB):
            for jt in range(JT):
                wr, ur = rw[jt]
                ps = psum.tile([128, NBS], F32, tag="ps", name=f"ps_r{jt}_{nb}")
                for k in range(0, KC, 2):
                    mm(
                        ps[:],
                        wr[:, k : k + 2, :],
                        xT8[:, k : k + 2, ds(nb * NBS, NBS)],
                        start=(k == 0),
                        stop=False,
                        reload_w=False,
                        perf_mode=DR,
                    )
                for k in range(0, KC, 2):
                    mm(
                        ps[:],
                        ur[:, k : k + 2, :],
                        hT8[:, k : k + 2, ds(nb * NBS, NBS)],
                        start=False,
                        stop=(k == KC - 2),
                        reload_w=False,
                        perf_mode=DR,
                    )
                rt = actp.tile([128, NBS], BF16, tag="act")
                nc.scalar.activation(
                    rt[:],
                    ps[:],
                    AFT.Sigmoid,
                    bias=bias_sb[:, 8 + jt : 9 + jt],
                    scale=INV_SCALE,
                )
                # rh in scaled fp8 for the DoubleRow A-matmul: (r*16)*h
                nc.vector.scalar_tensor_tensor(
                    rhT8[:, jt, ds(nb * NBS, NBS)],
                    rt[:],
                    SCALE_X,
                    hTb[:, jt, ds(nb * NBS, NBS)],
                    op0=ALU.mult,
                    op1=ALU.mult,
                )

        # ---- phase 2: z gate (fp8) + delta_h (bf16) + residual, per jt ----
        for jt in range(JT):
            wz = load_w("wz8", jt)
            uz = load_w("uz8", jt)
            vh = load_w("vhT", jt)
            at = load_w("at8", jt)
            psz = [
                psum.tile([128, NBS], F32, tag="ps", name=f"ps_z{jt}_{i}")
                for i in range(NB)
            ]
            gemm_pair_fp8(psz, wz, xT8, uz, hT8)
            # dh-pre: V_h@x in bf16 (V_h host-scaled by 4096) and A@(r*h) in
            # fp8 DoubleRow (scales 16*256) — separate PSUM groups (mixing
            # perf modes in one accumulation group faults the device), summed
            # on the DVE.
            psv = [
                psum.tile([128, NBS], F32, tag="ps", name=f"ps_v{jt}_{i}")
                for i in range(NB)
            ]
            vhs = []
            for nb in range(NB):
                for k in range(KC):
                    mm(
                        psv[nb][:],
                        vh[:, k, :],
                        xT[:, k, ds(nb * NBS, NBS)],
                        start=(k == 0),
                        stop=(k == KC - 1),
                        reload_w=False,
                    )
                t = tmpp.tile([128, NBS], F32, tag="vhs", name=f"vhs{jt}_{nb}")
                nc.vector.tensor_copy(t[:], psv[nb][:])
                vhs.append(t)
            psd = [
                psum.tile([128, NBS], F32, tag="ps", name=f"ps_d{jt}_{i}")
                for i in range(NB)
            ]
            for nb in range(NB):
                for k in range(0, KC, 2):
                    mm(
                        psd[nb][:],
                        at[:, k : k + 2, :],
                        rhT8[:, k : k + 2, ds(nb * NBS, NBS)],
                        start=(k == 0),
                        stop=(k == KC - 2),
                        reload_w=False,
                        perf_mode=DR,
                    )
            ot = outp.tile([128, B], F32, tag="out")
            for nb in range(NB):
                zt = actp.tile([128, NBS], BF16, tag="act")
                nc.scalar.activation(
                    zt[:],
                    psz[nb][:],
                    AFT.Sigmoid,
                    bias=bias_sb[:, jt : jt + 1],
                    scale=INV_SCALE,
                )
                dsum = tmpp.tile([128, NBS], F32, tag="dsum")
                nc.vector.tensor_add(dsum[:], psd[nb][:], vhs[nb][:])
                dt_ = actp.tile([128, NBS], BF16, tag="act")
                nc.scalar.activation(
                    dt_[:],
                    dsum[:],
                    AFT.Tanh,
                    bias=bias_sb[:, 16 + jt : 17 + jt],
                    scale=INV_SCALE,
                )
                zdh = tmpp.tile([128, NBS], F32, tag="zdh")
                nc.vector.tensor_mul(zdh[:], zt[:], dt_[:])
                # out = (z*dh) * eps + h
                nc.vector.scalar_tensor_tensor(
                    ot[:, ds(nb * NBS, NBS)],
                    zdh[:],
                    float(eps),
                    hTb[:, jt, ds(nb * NBS, NBS)],
                    op0=ALU.mult,
                    op1=ALU.add,
                )
                nc.sync.dma_start(
                    out=out_d[ts(jt, 128), ds(nb * NBS, NBS)],
                    in_=ot[:, ds(nb * NBS, NBS)],
                )

    nc.compile()
    return nc


def _get_nc(eps: float):
    key = float(eps)
    if key not in _nc_cache:
        _nc_cache[key] = _build(key)
    return _nc_cache[key]


def _block_weight(wT, dtype, scale=1.0):
    # [1024, 1024] (contraction-major) -> [jt, p, c, j] st. blk[jt,p,c,j] = wT[c*128+p, jt*128+j]
    blk = wT.reshape(KC, 128, JT, 128).transpose(2, 1, 0, 3)
    if scale != 1.0:
        blk = blk * scale
    return np.ascontiguousarray(blk).astype(dtype)


def _block_data(m):
    # per-core [B, 1024] -> [p, c, b] st. blk[p,c,b] = m[b, c*128+p]
    return np.ascontiguousarray(m.T.reshape(KC, 128, B).transpose(1, 0, 2))


def _block_data_nb(m):
    # per-core [B, 1024] -> [nb, p, c, nbs] batch-block-major
    blk = m.T.reshape(KC, 128, NB, NBS).transpose(2, 1, 0, 3)
    return np.ascontiguousarray(blk)


def _prep_in_maps(x, h_prev, W_z, b_z, U_z, W_r, b_r, U_r, V_h, b_h, W_h):
    BF = ml_dtypes.bfloat16
    F8 = ml_dtypes.float8_e4m3
    x16 = np.asarray(x, np.float32).astype(BF)
    h16 = np.asarray(h_prev, np.float32).astype(BF)
    x8 = (np.asarray(x, np.float32) * SCALE_X).astype(F8)
    h8 = (np.asarray(h_prev, np.float32) * SCALE_X).astype(F8)

    A = W_h - W_h.T - GAMMA * np.eye(H, dtype=np.float32)
    shared = {
        "wz8": _block_weight(W_z.T, F8, SCALE_W),
        "uz8": _block_weight(U_z.T, F8, SCALE_W),
        "wr8": _block_weight(W_r.T, F8, SCALE_W),
        "ur8": _block_weight(U_r.T, F8, SCALE_W),
        "at8": _block_weight(A.T, F8, SCALE_W),
        "vhT": _block_weight(V_h.T, BF, SCALE_X * SCALE_W),
        "biases": np.ascontiguousarray(
            np.concatenate(
                [
                    b_z.reshape(JT, 128).T,
                    b_r.reshape(JT, 128).T,
                    b_h.reshape(JT, 128).T,
                ],
                axis=1,
            ).astype(np.float32)
        ),
    }
    in_maps = []
    for c in range(N_CORES):
        sl = slice(c * B, (c + 1) * B)
        in_maps.append(
            {
                "xT": _block_data(x16[sl]),
                "hT": _block_data(h16[sl]),
                "xT8": _block_data_nb(x8[sl]),
                "hT8": _block_data_nb(h8[sl]),
                **shared,
            }
        )
    return in_maps


def run(inputs, trace=False):
    """Returns (full_output [16384,1024] f32, BassKernelResults)."""
    np_in = {k: np.asarray(v, np.float32) for k, v in inputs.items()}
    eps = float(np_in.pop("epsilon"))
    in_maps = _prep_in_maps(**np_in)
    nc = _get_nc(eps)
    res = run_bass_kernel_spmd(
        nc, in_maps, core_ids=list(range(N_CORES)), trace=trace
    )
    out = np.empty((BATCH, H), np.float32)
    for c in range(N_CORES):
        out[c * B : (c + 1) * B, :] = res.results[c]["out"].T
    return out, res


def kernel(**inputs) -> np.ndarray:
    out, _ = run(inputs, trace=False)
    return out


# revision 75
# speedup vs baseline: 1.0838x; 1.0576x over previous
"""AGRU cell (antisymmetric GRU) forward on 8 TRN2 NeuronCores.

Data-parallel: batch 16384 is sharded 2048 rows/core; the six 1024x1024
weight matrices are replicated. No cross-core communication.

Everything on-device is computed in "hidden-major" (transposed) layout:
    zT = sigmoid(Wz @ xT + Uz @ hT + bz)        [H, B]
    rT = sigmoid(Wr @ xT + Ur @ hT + br)
    rhT = rT * hT
    dhT = tanh(Vh @ xT + A @ rhT + bh)
    outT = hT + eps * zT * dhT
so every matmul has the (pre-transposed, host-prepared) weight tile as the
stationary operand and xT/hT/rhT as the moving operand, and nothing ever
needs an on-device transpose.  The host transposes each core's [1024, 2048]
result back when assembling the full output.

Precision: the sigmoid-gate GEMMs (z, r) and the A@(r*h) term run in fp8
e4m3 with DoubleRow perf mode (2 contraction rows/cycle); inputs are
pre-scaled (x*16, W*256) and the scale is folded back in the activation
(scale=1/4096), which also applies the per-partition bias.  V_h@x runs in
bf16 (tanh passes its error through undamped, so it gets the accurate
path).  All accumulation is fp32 in PSUM; the residual h + eps*z*dh is
fp32 on the vector engine.  Measured rel err vs the fp32 reference: 9.1e-3.
"""

import sys

sys.path.insert(0, "/opt/trn_rl_repo")

import numpy as np
import ml_dtypes

from contextlib import ExitStack

import concourse.bass as bass
import concourse.mybir as mybir
from concourse import bacc, tile
from concourse.bass import ds, ts
from concourse.bass_utils import run_bass_kernel_spmd

BF16 = mybir.dt.bfloat16
FP8 = mybir.dt.float8e4
F32 = mybir.dt.float32
AFT = mybir.ActivationFunctionType
ALU = mybir.AluOpType
DR = mybir.MatmulPerfMode.DoubleRow

# fp8 pre-scaling for the sigmoid-gate GEMMs (z, r): data*16, weights*256,
# compensated by activation scale 1/(16*256).
SCALE_X = 16.0
SCALE_W = 256.0
INV_SCALE = 1.0 / (SCALE_X * SCALE_W)

# V_h@x contraction chunks 0..VH_FP8-1 run in fp8 DoubleRow (folded into the
# A-path PSUM group, same mode/scale); chunks VH_FP8..7 run in bf16. fp8 dot
# error scales with sqrt(VH_FP8/8): 0 -> rel err 9.1e-3, 4 -> ~1.3e-2,
# 8 -> 1.5e-2 (measured). 4 keeps a ~1.5x margin under the 2e-2 gate.
VH_FP8 = 4

N_CORES = 8
BATCH = 16384
B = BATCH // N_CORES  # per-core batch shard (2048)
H = 1024  # hidden == input size
KC = H // 128  # contraction chunks (8)
JT = H // 128  # output row tiles (8)
NB = 4  # moving-dim (batch) blocks per psum bank
NBS = B // NB  # 512 columns per matmul
GAMMA = 0.01

_nc_cache = {}


def _build(eps: float):
    """Build + compile the single-core Tile program (same graph on all cores)."""
    nc = bacc.Bacc("TRN2", target_bir_lowering=False, debug=False)

    xT_d = nc.dram_tensor("xT", [128, KC, B], BF16, kind="ExternalInput")
    hT_d = nc.dram_tensor("hT", [128, KC, B], BF16, kind="ExternalInput")
    # fp8 activations arrive batch-block-major so phase 1 can start on the
    # first 512-column block instead of waiting for the full stream.
    xT8_d = nc.dram_tensor("xT8", [NB, 128, KC, NBS], FP8, kind="ExternalInput")
    hT8_d = nc.dram_tensor("hT8", [NB, 128, KC, NBS], FP8, kind="ExternalInput")
    w_d = {
        name: nc.dram_tensor(name, [JT, 128, KC, 128], BF16, kind="ExternalInput")
        for name in ["vhT"]
    }
    w8_d = {
        name: nc.dram_tensor(name, [JT, 128, KC, 128], FP8, kind="ExternalInput")
        for name in ["wz8", "uz8", "wr8", "ur8", "at8", "vh8"]
    }
    bias_d = nc.dram_tensor("biases", [128, 24], F32, kind="ExternalInput")
    out_d = nc.dram_tensor("out", [H, B], F32, kind="ExternalOutput")

    with tile.TileContext(nc) as tc, ExitStack() as ctx:
        singles = ctx.enter_context(tc.tile_pool(name="singles", bufs=1))
        wpool = ctx.enter_context(tc.tile_pool(name="wpool", bufs=7))
        rwpool = ctx.enter_context(tc.tile_pool(name="rwpool", bufs=16))
        psum = ctx.enter_context(tc.tile_pool(name="psum", bufs=8, space="PSUM"))
        actp = ctx.enter_context(tc.tile_pool(name="actp", bufs=10))
        tmpp = ctx.enter_context(tc.tile_pool(name="tmpp", bufs=4))
        outp = ctx.enter_context(tc.tile_pool(name="outp", bufs=2))

        xT = singles.tile([128, KC, B], BF16)
        hTb = singles.tile([128, KC, B], BF16)
        xT8 = singles.tile([128, KC, B], FP8)
        hT8 = singles.tile([128, KC, B], FP8)
        rhT8 = singles.tile([128, KC, B], FP8)
        bias_sb = singles.tile([128, 24], F32)

        def load_w(name, jt):
            # gpsimd (SWDGE): keeps weight blocks off the Sync HWDGE queue,
            # which is busy issuing the bulk x/h stream.
            fp8 = name in w8_d
            w = wpool.tile([128, KC, 128], FP8 if fp8 else BF16, tag="w")
            nc.gpsimd.dma_start(out=w[:], in_=(w8_d[name] if fp8 else w_d[name])[jt])
            return w

        def load_rw(name, jt):
            # phase-1 weights stay resident across all 4 batch-block passes
            w = rwpool.tile([128, KC, 128], FP8, tag="rw")
            nc.sync.dma_start(out=w[:], in_=w8_d[name][jt])
            return w

        # Issue order on the sync queue matches consumption order: jt0's
        # r-weights, the first x8/h8 batch block, then the remaining
        # r-weights interleaved with the hTb chunks (pass nb0 consumes
        # rw[jt] and hTb[:,jt,:] in lockstep, one pair every ~3.4us).
        rw = [(load_rw("wr8", 0), load_rw("ur8", 0))]
        # first batch block split by contraction chunk-pairs, x before h,
        # matching the k-sweep consumption order of the first matmul group
        for kp in range(0, KC, 2):
            nc.sync.dma_start(
                out=xT8[:, kp : kp + 2, ds(0, NBS)], in_=xT8_d[0][:, kp : kp + 2, :]
            )
        for kp in range(0, KC, 2):
            nc.sync.dma_start(
                out=hT8[:, kp : kp + 2, ds(0, NBS)], in_=hT8_d[0][:, kp : kp + 2, :]
            )
        nc.sync.dma_start(out=bias_sb[:], in_=bias_d[:])
        nc.sync.dma_start(out=hTb[:, 0, :], in_=hT_d[:, 0, :])
        for jt in range(1, JT):
            rw.append((load_rw("wr8", jt), load_rw("ur8", jt)))
            nc.sync.dma_start(out=hTb[:, jt, :], in_=hT_d[:, jt, :])
        # remaining x8/h8 blocks (needed from pass nb1 on) and bf16 x (phase 2)
        for nb in range(1, NB):
            nc.sync.dma_start(out=xT8[:, :, ds(nb * NBS, NBS)], in_=xT8_d[nb])
            nc.sync.dma_start(out=hT8[:, :, ds(nb * NBS, NBS)], in_=hT8_d[nb])
        for c in range(VH_FP8, KC):
            nc.sync.dma_start(out=xT[:, c, :], in_=xT_d[:, c, :])

        def mm(psum_ap, w_ap, rhs_ap, start, stop, reload_w, perf_mode=None):
            # (LDWEIGHTS issue is left to legalization; measured to overlap
            # the matmul stream fully, so no manual dedupe/ordering needed.)
            return nc.tensor.matmul(
                psum_ap, w_ap, rhs_ap, start=start, stop=stop, perf_mode=perf_mode
            )

        def gemm_pair_fp8(psums, wA, rhsA, wB, rhsB):
            # fp8 DoubleRow, nb-outer: each PSUM bank completes (and is
            # evicted by the ACT) while later banks still accumulate.
            for nb in range(NB):
                for k in range(0, KC, 2):
                    mm(
                        psums[nb][:],
                        wA[:, k : k + 2, :],
                        rhsA[:, k : k + 2, ds(nb * NBS, NBS)],
                        start=(k == 0),
                        stop=False,
                        reload_w=False,
                        perf_mode=DR,
                    )
                for k in range(0, KC, 2):
                    mm(
                        psums[nb][:],
                        wB[:, k : k + 2, :],
                        rhsB[:, k : k + 2, ds(nb * NBS, NBS)],
                        start=False,
                        stop=(k == KC - 2),
                        reload_w=False,
                        perf_mode=DR,
                    )

        # ---- phase 1: r gate (hidden-major, fp8), rhT = sigmoid(...) * hT ----
        # Batch-block-outer, jt-inner: the first pass needs only the first
        # 512-column x8/h8 block plus the r weights, so the PE starts early
        # and never waits on the bulk DMA. The 8 jt groups of one pass
        # exactly fill the 8 PSUM banks.
        for nb in range(NB):
            for jt in range(JT):
                wr, ur = rw[jt]
                ps = psum.tile([128, NBS], F32, tag="ps", name=f"ps_r{jt}_{nb}")
                for k in range(0, KC, 2):
                    mm(
                        ps[:],
                        wr[:, k : k + 2, :],
                        xT8[:, k : k + 2, ds(nb * NBS, NBS)],
                        start=(k == 0),
                        stop=False,
                        reload_w=False,
                        perf_mode=DR,
                    )
                for k in range(0, KC, 2):
                    mm(
                        ps[:],
                        ur[:, k : k + 2, :],
                        hT8[:, k : k + 2, ds(nb * NBS, NBS)],
                        start=False,
                        stop=(k == KC - 2),
                        reload_w=False,
                        perf_mode=DR,
                    )
                rt = actp.tile([128, NBS], BF16, tag="act")
                nc.scalar.activation(
                    rt[:],
                    ps[:],
                    AFT.Sigmoid,
                    bias=bias_sb[:, 8 + jt : 9 + jt],
                    scale=INV_SCALE,
                )
                # rh in scaled fp8 for the DoubleRow A-matmul: (r*16)*h
                nc.vector.scalar_tensor_tensor(
                    rhT8[:, jt, ds(nb * NBS, NBS)],
                    rt[:],
                    SCALE_X,
                    hTb[:, jt, ds(nb * NBS, NBS)],
                    op0=ALU.mult,
                    op1=ALU.mult,
                )

        # ---- phase 2: z gate (fp8) + delta_h (bf16) + residual, per jt ----
        for jt in range(JT):
            wz = load_w("wz8", jt)
            uz = load_w("uz8", jt)
            vh = load_w("vhT", jt)
            vh8 = load_w("vh8", jt)
            at = load_w("at8", jt)
            psz = [
                psum.tile([128, NBS], F32, tag="ps", name=f"ps_z{jt}_{i}")
                for i in range(NB)
            ]
            gemm_pair_fp8(psz, wz, xT8, uz, hT8)
            # dh-pre: V_h@x chunks >= VH_FP8 in bf16 (V_h host-scaled by
            # 4096) in their own PSUM group (mixing perf modes in one
            # accumulation group faults the device); V_h@x chunks < VH_FP8
            # plus A@(r*h), all fp8 DoubleRow at the same 16*256 scale,
            # share the second group. DVE adds the two.
            psv = [
                psum.tile([128, NBS], F32, tag="ps", name=f"ps_v{jt}_{i}")
                for i in range(NB)
            ]
            vhs = []
            for nb in range(NB):
                for k in range(VH_FP8, KC):
                    mm(
                        psv[nb][:],
                        vh[:, k, :],
                        xT[:, k, ds(nb * NBS, NBS)],
                        start=(k == VH_FP8),
                        stop=(k == KC - 1),
                        reload_w=False,
                    )
                t = tmpp.tile([128, NBS], F32, tag="vhs", name=f"vhs{jt}_{nb}")
                nc.vector.tensor_copy(t[:], psv[nb][:])
                vhs.append(t)
            psd = [
                psum.tile([128, NBS], F32, tag="ps", name=f"ps_d{jt}_{i}")
                for i in range(NB)
            ]
            for nb in range(NB):
                for k in range(0, VH_FP8, 2):
                    mm(
                        psd[nb][:],
                        vh8[:, k : k + 2, :],
                        xT8[:, k : k + 2, ds(nb * NBS, NBS)],
                        start=(k == 0),
                        stop=False,
                        reload_w=False,
                        perf_mode=DR,
                    )
                for k in range(0, KC, 2):
                    mm(
                        psd[nb][:],
                        at[:, k : k + 2, :],
                        rhT8[:, k : k + 2, ds(nb * NBS, NBS)],
                        start=False,
                        stop=(k == KC - 2),
                        reload_w=False,
                        perf_mode=DR,
                    )
            ot = outp.tile([128, B], F32, tag="out")
            for nb in range(NB):
                zt = actp.tile([128, NBS], BF16, tag="act")
                nc.scalar.activation(
                    zt[:],
                    psz[nb][:],
                    AFT.Sigmoid,
                    bias=bias_sb[:, jt : jt + 1],
                    scale=INV_SCALE,
                )
                dsum = tmpp.tile([128, NBS], F32, tag="dsum")
                nc.vector.tensor_add(dsum[:], psd[nb][:], vhs[nb][:])
                dt_ = actp.tile([128, NBS], BF16, tag="act")
                nc.scalar.activation(
                    dt_[:],
                    dsum[:],
                    AFT.Tanh,
                    bias=bias_sb[:, 16 + jt : 17 + jt],
                    scale=INV_SCALE,
                )
                zdh = tmpp.tile([128, NBS], F32, tag="zdh")
                nc.vector.tensor_mul(zdh[:], zt[:], dt_[:])
                # out = (z*dh) * eps + h
                nc.vector.scalar_tensor_tensor(
                    ot[:, ds(nb * NBS, NBS)],
                    zdh[:],
                    float(eps),
                    hTb[:, jt, ds(nb * NBS, NBS)],
                    op0=ALU.mult,
                    op1=ALU.add,
                )
                nc.sync.dma_start(
                    out=out_d[ts(jt, 128), ds(nb * NBS, NBS)],
                    in_=ot[:, ds(nb * NBS, NBS)],
                )

    nc.compile()
    return nc


def _get_nc(eps: float):
    key = float(eps)
    if key not in _nc_cache:
        _nc_cache[key] = _build(key)
    return _nc_cache[key]


def _block_weight(wT, dtype, scale=1.0):
    # [1024, 1024] (contraction-major) -> [jt, p, c, j] st. blk[jt,p,c,j] = wT[c*128+p, jt*128+j]
    blk = wT.reshape(KC, 128, JT, 128).transpose(2, 1, 0, 3)
    if scale != 1.0:
        blk = blk * scale
    return np.ascontiguousarray(blk).astype(dtype)


def _block_data(m):
    # per-core [B, 1024] -> [p, c, b] st. blk[p,c,b] = m[b, c*128+p]
    return np.ascontiguousarray(m.T.reshape(KC, 128, B).transpose(1, 0, 2))


def _block_data_nb(m):
    # per-core [B, 1024] -> [nb, p, c, nbs] batch-block-major
    blk = m.T.reshape(KC, 128, NB, NBS).transpose(2, 1, 0, 3)
    return np.ascontiguousarray(blk)


def _prep_in_maps(x, h_prev, W_z, b_z, U_z, W_r, b_r, U_r, V_h, b_h, W_h):
    BF = ml_dtypes.bfloat16
    F8 = ml_dtypes.float8_e4m3
    x16 = np.asarray(x, np.float32).astype(BF)
    h16 = np.asarray(h_prev, np.float32).astype(BF)
    x8 = (np.asarray(x, np.float32) * SCALE_X).astype(F8)
    h8 = (np.asarray(h_prev, np.float32) * SCALE_X).astype(F8)

    A = W_h - W_h.T - GAMMA * np.eye(H, dtype=np.float32)
    shared = {
        "wz8": _block_weight(W_z.T, F8, SCALE_W),
        "uz8": _block_weight(U_z.T, F8, SCALE_W),
        "wr8": _block_weight(W_r.T, F8, SCALE_W),
        "ur8": _block_weight(U_r.T, F8, SCALE_W),
        "at8": _block_weight(A.T, F8, SCALE_W),
        "vh8": _block_weight(V_h.T, F8, SCALE_W),
        "vhT": _block_weight(V_h.T, BF, SCALE_X * SCALE_W),
        "biases": np.ascontiguousarray(
            np.concatenate(
                [
                    b_z.reshape(JT, 128).T,
                    b_r.reshape(JT, 128).T,
                    b_h.reshape(JT, 128).T,
                ],
                axis=1,
            ).astype(np.float32)
        ),
    }
    in_maps = []
    for c in range(N_CORES):
        sl = slice(c * B, (c + 1) * B)
        in_maps.append(
            {
                "xT": _block_data(x16[sl]),
                "hT": _block_data(h16[sl]),
                "xT8": _block_data_nb(x8[sl]),
                "hT8": _block_data_nb(h8[sl]),
                **shared,
            }
        )
    return in_maps


def run(inputs, trace=False):
    """Returns (full_output [16384,1024] f32, BassKernelResults)."""
    np_in = {k: np.asarray(v, np.float32) for k, v in inputs.items()}
    eps = float(np_in.pop("epsilon"))
    in_maps = _prep_in_maps(**np_in)
    nc = _get_nc(eps)
    res = run_bass_kernel_spmd(
        nc, in_maps, core_ids=list(range(N_CORES)), trace=trace
    )
    out = np.empty((BATCH, H), np.float32)
    for c in range(N_CORES):
        out[c * B : (c + 1) * B, :] = res.results[c]["out"].T
    return out, res


def kernel(**inputs) -> np.ndarray:
    out, _ = run(inputs, trace=False)
    return out


# revision 76
# speedup vs baseline: 1.0853x; 1.0013x over previous
"""AGRU cell (antisymmetric GRU) forward on 8 TRN2 NeuronCores.

Data-parallel: batch 16384 is sharded 2048 rows/core; the six 1024x1024
weight matrices are replicated. No cross-core communication.

Everything on-device is computed in "hidden-major" (transposed) layout:
    zT = sigmoid(Wz @ xT + Uz @ hT + bz)        [H, B]
    rT = sigmoid(Wr @ xT + Ur @ hT + br)
    rhT = rT * hT
    dhT = tanh(Vh @ xT + A @ rhT + bh)
    outT = hT + eps * zT * dhT
so every matmul has the (pre-transposed, host-prepared) weight tile as the
stationary operand and xT/hT/rhT as the moving operand, and nothing ever
needs an on-device transpose.  The host transposes each core's [1024, 2048]
result back when assembling the full output.

Precision: the sigmoid-gate GEMMs (z, r) and the A@(r*h) term run in fp8
e4m3 with DoubleRow perf mode (2 contraction rows/cycle); inputs are
pre-scaled (x*16, W*256) and the scale is folded back in the activation
(scale=1/4096), which also applies the per-partition bias.  V_h@x (whose
error tanh passes through undamped) is split: half its contraction chunks
run in fp8 DoubleRow inside the A-path PSUM group, half in bf16 in a
separate group (VH_FP8 tunes the speed/accuracy dial).  All accumulation
is fp32 in PSUM; the residual h + eps*z*dh is fp32 on the vector engine.
Measured rel err vs the fp32 reference: 1.26e-2 (gate: 2e-2).
"""

import sys

sys.path.insert(0, "/opt/trn_rl_repo")

import numpy as np
import ml_dtypes

from contextlib import ExitStack

import concourse.bass as bass
import concourse.mybir as mybir
from concourse import bacc, tile
from concourse.bass import ds, ts
from concourse.bass_utils import run_bass_kernel_spmd

BF16 = mybir.dt.bfloat16
FP8 = mybir.dt.float8e4
F32 = mybir.dt.float32
AFT = mybir.ActivationFunctionType
ALU = mybir.AluOpType
DR = mybir.MatmulPerfMode.DoubleRow

# fp8 pre-scaling for the sigmoid-gate GEMMs (z, r): data*16, weights*256,
# compensated by activation scale 1/(16*256).
SCALE_X = 16.0
SCALE_W = 256.0
INV_SCALE = 1.0 / (SCALE_X * SCALE_W)

# V_h@x contraction chunks 0..VH_FP8-1 run in fp8 DoubleRow (folded into the
# A-path PSUM group, same mode/scale); chunks VH_FP8..7 run in bf16. fp8 dot
# error scales with sqrt(VH_FP8/8): 0 -> rel err 9.1e-3, 4 -> ~1.3e-2,
# 8 -> 1.5e-2 (measured). 4 keeps a ~1.5x margin under the 2e-2 gate.
VH_FP8 = 4

N_CORES = 8
BATCH = 16384
B = BATCH // N_CORES  # per-core batch shard (2048)
H = 1024  # hidden == input size
KC = H // 128  # contraction chunks (8)
JT = H // 128  # output row tiles (8)
NB = 4  # moving-dim (batch) blocks per psum bank
NBS = B // NB  # 512 columns per matmul
GAMMA = 0.01

_nc_cache = {}


def _build(eps: float):
    """Build + compile the single-core Tile program (same graph on all cores)."""
    nc = bacc.Bacc("TRN2", target_bir_lowering=False, debug=False)

    xT_d = nc.dram_tensor("xT", [128, KC, B], BF16, kind="ExternalInput")
    hT_d = nc.dram_tensor("hT", [128, KC, B], BF16, kind="ExternalInput")
    # fp8 activations arrive batch-block-major so phase 1 can start on the
    # first 512-column block instead of waiting for the full stream.
    xT8_d = nc.dram_tensor("xT8", [NB, 128, KC, NBS], FP8, kind="ExternalInput")
    hT8_d = nc.dram_tensor("hT8", [NB, 128, KC, NBS], FP8, kind="ExternalInput")
    w_d = {
        name: nc.dram_tensor(name, [JT, 128, KC, 128], BF16, kind="ExternalInput")
        for name in ["vhT"]
    }
    w8_d = {
        name: nc.dram_tensor(name, [JT, 128, KC, 128], FP8, kind="ExternalInput")
        for name in ["wz8", "uz8", "wr8", "ur8", "at8", "vh8"]
    }
    bias_d = nc.dram_tensor("biases", [128, 24], F32, kind="ExternalInput")
    out_d = nc.dram_tensor("out", [H, B], F32, kind="ExternalOutput")

    with tile.TileContext(nc) as tc, ExitStack() as ctx:
        singles = ctx.enter_context(tc.tile_pool(name="singles", bufs=1))
        wpool = ctx.enter_context(tc.tile_pool(name="wpool", bufs=7))
        rwpool = ctx.enter_context(tc.tile_pool(name="rwpool", bufs=16))
        psum = ctx.enter_context(tc.tile_pool(name="psum", bufs=8, space="PSUM"))
        actp = ctx.enter_context(tc.tile_pool(name="actp", bufs=10))
        tmpp = ctx.enter_context(tc.tile_pool(name="tmpp", bufs=4))
        outp = ctx.enter_context(tc.tile_pool(name="outp", bufs=2))

        xT = singles.tile([128, KC, B], BF16)
        hTb = singles.tile([128, KC, B], BF16)
        xT8 = singles.tile([128, KC, B], FP8)
        hT8 = singles.tile([128, KC, B], FP8)
        rhT8 = singles.tile([128, KC, B], FP8)
        bias_sb = singles.tile([128, 24], F32)

        def load_w(name, jt):
            # gpsimd (SWDGE): keeps weight blocks off the Sync HWDGE queue,
            # which is busy issuing the bulk x/h stream.
            fp8 = name in w8_d
            w = wpool.tile([128, KC, 128], FP8 if fp8 else BF16, tag="w")
            nc.gpsimd.dma_start(out=w[:], in_=(w8_d[name] if fp8 else w_d[name])[jt])
            return w

        def load_rw(name, jt):
            # phase-1 weights stay resident across all 4 batch-block passes
            w = rwpool.tile([128, KC, 128], FP8, tag="rw")
            nc.sync.dma_start(out=w[:], in_=w8_d[name][jt])
            return w

        # Issue order on the sync queue matches consumption order: jt0's
        # r-weights, the first x8/h8 batch block, then the remaining
        # r-weights interleaved with the hTb chunks (pass nb0 consumes
        # rw[jt] and hTb[:,jt,:] in lockstep, one pair every ~3.4us).
        rw = [(load_rw("wr8", 0), load_rw("ur8", 0))]
        # first batch block split by contraction chunk-pairs, x before h,
        # matching the k-sweep consumption order of the first matmul group
        for kp in range(0, KC, 2):
            nc.sync.dma_start(
                out=xT8[:, kp : kp + 2, ds(0, NBS)], in_=xT8_d[0][:, kp : kp + 2, :]
            )
        for kp in range(0, KC, 2):
            nc.sync.dma_start(
                out=hT8[:, kp : kp + 2, ds(0, NBS)], in_=hT8_d[0][:, kp : kp + 2, :]
            )
        nc.sync.dma_start(out=bias_sb[:], in_=bias_d[:])
        nc.sync.dma_start(out=hTb[:, 0, :], in_=hT_d[:, 0, :])
        for jt in range(1, JT):
            rw.append((load_rw("wr8", jt), load_rw("ur8", jt)))
            nc.sync.dma_start(out=hTb[:, jt, :], in_=hT_d[:, jt, :])
        # remaining x8/h8 blocks (needed from pass nb1 on) and bf16 x (phase 2)
        for nb in range(1, NB):
            nc.sync.dma_start(out=xT8[:, :, ds(nb * NBS, NBS)], in_=xT8_d[nb])
            nc.sync.dma_start(out=hT8[:, :, ds(nb * NBS, NBS)], in_=hT8_d[nb])
        for c in range(VH_FP8, KC):
            nc.sync.dma_start(out=xT[:, c, :], in_=xT_d[:, c, :])

        def mm(psum_ap, w_ap, rhs_ap, start, stop, reload_w, perf_mode=None):
            # (LDWEIGHTS issue is left to legalization; measured to overlap
            # the matmul stream fully, so no manual dedupe/ordering needed.)
            return nc.tensor.matmul(
                psum_ap, w_ap, rhs_ap, start=start, stop=stop, perf_mode=perf_mode
            )

        def gemm_pair_fp8(psums, wA, rhsA, wB, rhsB):
            # fp8 DoubleRow, nb-outer: each PSUM bank completes (and is
            # evicted by the ACT) while later banks still accumulate.
            for nb in range(NB):
                for k in range(0, KC, 2):
                    mm(
                        psums[nb][:],
                        wA[:, k : k + 2, :],
                        rhsA[:, k : k + 2, ds(nb * NBS, NBS)],
                        start=(k == 0),
                        stop=False,
                        reload_w=False,
                        perf_mode=DR,
                    )
                for k in range(0, KC, 2):
                    mm(
                        psums[nb][:],
                        wB[:, k : k + 2, :],
                        rhsB[:, k : k + 2, ds(nb * NBS, NBS)],
                        start=False,
                        stop=(k == KC - 2),
                        reload_w=False,
                        perf_mode=DR,
                    )

        # ---- phase 1: r gate (hidden-major, fp8), rhT = sigmoid(...) * hT ----
        # Batch-block-outer, jt-inner: the first pass needs only the first
        # 512-column x8/h8 block plus the r weights, so the PE starts early
        # and never waits on the bulk DMA. The 8 jt groups of one pass
        # exactly fill the 8 PSUM banks.
        for nb in range(NB):
            for jt in range(JT):
                wr, ur = rw[jt]
                ps = psum.tile([128, NBS], F32, tag="ps", name=f"ps_r{jt}_{nb}")
                for k in range(0, KC, 2):
                    mm(
                        ps[:],
                        wr[:, k : k + 2, :],
                        xT8[:, k : k + 2, ds(nb * NBS, NBS)],
                        start=(k == 0),
                        stop=False,
                        reload_w=False,
                        perf_mode=DR,
                    )
                for k in range(0, KC, 2):
                    mm(
                        ps[:],
                        ur[:, k : k + 2, :],
                        hT8[:, k : k + 2, ds(nb * NBS, NBS)],
                        start=False,
                        stop=(k == KC - 2),
                        reload_w=False,
                        perf_mode=DR,
                    )
                rt = actp.tile([128, NBS], BF16, tag="act")
                nc.scalar.activation(
                    rt[:],
                    ps[:],
                    AFT.Sigmoid,
                    bias=bias_sb[:, 8 + jt : 9 + jt],
                    scale=INV_SCALE,
                )
                # rh in scaled fp8 for the DoubleRow A-matmul: (r*16)*h
                nc.vector.scalar_tensor_tensor(
                    rhT8[:, jt, ds(nb * NBS, NBS)],
                    rt[:],
                    SCALE_X,
                    hTb[:, jt, ds(nb * NBS, NBS)],
                    op0=ALU.mult,
                    op1=ALU.mult,
                )

        # ---- phase 2: z gate (fp8) + delta_h (bf16) + residual, per jt ----
        for jt in range(JT):
            wz = load_w("wz8", jt)
            uz = load_w("uz8", jt)
            vh = load_w("vhT", jt)
            vh8 = load_w("vh8", jt)
            at = load_w("at8", jt)
            psz = [
                psum.tile([128, NBS], F32, tag="ps", name=f"ps_z{jt}_{i}")
                for i in range(NB)
            ]
            gemm_pair_fp8(psz, wz, xT8, uz, hT8)
            # dh-pre: V_h@x chunks >= VH_FP8 in bf16 (V_h host-scaled by
            # 4096) in their own PSUM group (mixing perf modes in one
            # accumulation group faults the device); V_h@x chunks < VH_FP8
            # plus A@(r*h), all fp8 DoubleRow at the same 16*256 scale,
            # share the second group. DVE adds the two.
            psv = [
                psum.tile([128, NBS], F32, tag="ps", name=f"ps_v{jt}_{i}")
                for i in range(NB)
            ]
            vhs = []
            for nb in range(NB):
                for k in range(VH_FP8, KC):
                    mm(
                        psv[nb][:],
                        vh[:, k, :],
                        xT[:, k, ds(nb * NBS, NBS)],
                        start=(k == VH_FP8),
                        stop=(k == KC - 1),
                        reload_w=False,
                    )
                t = tmpp.tile([128, NBS], F32, tag="vhs", name=f"vhs{jt}_{nb}")
                nc.vector.tensor_copy(t[:], psv[nb][:])
                vhs.append(t)
            psd = [
                psum.tile([128, NBS], F32, tag="ps", name=f"ps_d{jt}_{i}")
                for i in range(NB)
            ]
            for nb in range(NB):
                for k in range(0, VH_FP8, 2):
                    mm(
                        psd[nb][:],
                        vh8[:, k : k + 2, :],
                        xT8[:, k : k + 2, ds(nb * NBS, NBS)],
                        start=(k == 0),
                        stop=False,
                        reload_w=False,
                        perf_mode=DR,
                    )
                for k in range(0, KC, 2):
                    mm(
                        psd[nb][:],
                        at[:, k : k + 2, :],
                        rhT8[:, k : k + 2, ds(nb * NBS, NBS)],
                        start=False,
                        stop=(k == KC - 2),
                        reload_w=False,
                        perf_mode=DR,
                    )
            ot = outp.tile([128, B], F32, tag="out")
            for nb in range(NB):
                zt = actp.tile([128, NBS], BF16, tag="act")
                nc.scalar.activation(
                    zt[:],
                    psz[nb][:],
                    AFT.Sigmoid,
                    bias=bias_sb[:, jt : jt + 1],
                    scale=INV_SCALE,
                )
                dsum = tmpp.tile([128, NBS], F32, tag="dsum")
                nc.vector.tensor_add(dsum[:], psd[nb][:], vhs[nb][:])
                dt_ = actp.tile([128, NBS], BF16, tag="act")
                nc.scalar.activation(
                    dt_[:],
                    dsum[:],
                    AFT.Tanh,
                    bias=bias_sb[:, 16 + jt : 17 + jt],
                    scale=INV_SCALE,
                )
                zdh = tmpp.tile([128, NBS], F32, tag="zdh")
                nc.vector.tensor_mul(zdh[:], zt[:], dt_[:])
                # out = (z*dh) * eps + h
                nc.vector.scalar_tensor_tensor(
                    ot[:, ds(nb * NBS, NBS)],
                    zdh[:],
                    float(eps),
                    hTb[:, jt, ds(nb * NBS, NBS)],
                    op0=ALU.mult,
                    op1=ALU.add,
                )
                nc.sync.dma_start(
                    out=out_d[ts(jt, 128), ds(nb * NBS, NBS)],
                    in_=ot[:, ds(nb * NBS, NBS)],
                )

    nc.compile()
    return nc


def _get_nc(eps: float):
    key = float(eps)
    if key not in _nc_cache:
        _nc_cache[key] = _build(key)
    return _nc_cache[key]


def _block_weight(wT, dtype, scale=1.0):
    # [1024, 1024] (contraction-major) -> [jt, p, c, j] st. blk[jt,p,c,j] = wT[c*128+p, jt*128+j]
    blk = wT.reshape(KC, 128, JT, 128).transpose(2, 1, 0, 3)
    if scale != 1.0:
        blk = blk * scale
    return np.ascontiguousarray(blk).astype(dtype)


def _block_data(m):
    # per-core [B, 1024] -> [p, c, b] st. blk[p,c,b] = m[b, c*128+p]
    return np.ascontiguousarray(m.T.reshape(KC, 128, B).transpose(1, 0, 2))


def _block_data_nb(m):
    # per-core [B, 1024] -> [nb, p, c, nbs] batch-block-major
    blk = m.T.reshape(KC, 128, NB, NBS).transpose(2, 1, 0, 3)
    return np.ascontiguousarray(blk)


def _prep_in_maps(x, h_prev, W_z, b_z, U_z, W_r, b_r, U_r, V_h, b_h, W_h):
    BF = ml_dtypes.bfloat16
    F8 = ml_dtypes.float8_e4m3
    x16 = np.asarray(x, np.float32).astype(BF)
    h16 = np.asarray(h_prev, np.float32).astype(BF)
    x8 = (np.asarray(x, np.float32) * SCALE_X).astype(F8)
    h8 = (np.asarray(h_prev, np.float32) * SCALE_X).astype(F8)

    A = W_h - W_h.T - GAMMA * np.eye(H, dtype=np.float32)
    shared = {
        "wz8": _block_weight(W_z.T, F8, SCALE_W),
        "uz8": _block_weight(U_z.T, F8, SCALE_W),
        "wr8": _block_weight(W_r.T, F8, SCALE_W),
        "ur8": _block_weight(U_r.T, F8, SCALE_W),
        "at8": _block_weight(A.T, F8, SCALE_W),
        "vh8": _block_weight(V_h.T, F8, SCALE_W),
        "vhT": _block_weight(V_h.T, BF, SCALE_X * SCALE_W),
        "biases": np.ascontiguousarray(
            np.concatenate(
                [
                    b_z.reshape(JT, 128).T,
                    b_r.reshape(JT, 128).T,
                    b_h.reshape(JT, 128).T,
                ],
                axis=1,
            ).astype(np.float32)
        ),
    }
    in_maps = []
    for c in range(N_CORES):
        sl = slice(c * B, (c + 1) * B)
        in_maps.append(
            {
                "xT": _block_data(x16[sl]),
                "hT": _block_data(h16[sl]),
                "xT8": _block_data_nb(x8[sl]),
                "hT8": _block_data_nb(h8[sl]),
                **shared,
            }
        )
    return in_maps


def run(inputs, trace=False):
    """Returns (full_output [16384,1024] f32, BassKernelResults)."""
    np_in = {k: np.asarray(v, np.float32) for k, v in inputs.items()}
    eps = float(np_in.pop("epsilon"))
    in_maps = _prep_in_maps(**np_in)
    nc = _get_nc(eps)
    res = run_bass_kernel_spmd(
        nc, in_maps, core_ids=list(range(N_CORES)), trace=trace
    )
    out = np.empty((BATCH, H), np.float32)
    for c in range(N_CORES):
        out[c * B : (c + 1) * B, :] = res.results[c]["out"].T
    return out, res


def kernel(**inputs) -> np.ndarray:
    out, _ = run(inputs, trace=False)
    return out


# revision 77
# speedup vs baseline: 1.1086x; 1.0215x over previous
"""AGRU cell (antisymmetric GRU) forward on 8 TRN2 NeuronCores.

Data-parallel: batch 16384 is sharded 2048 rows/core; the six 1024x1024
weight matrices are replicated. No cross-core communication.

Everything on-device is computed in "hidden-major" (transposed) layout:
    zT = sigmoid(Wz @ xT + Uz @ hT + bz)        [H, B]
    rT = sigmoid(Wr @ xT + Ur @ hT + br)
    rhT = rT * hT
    dhT = tanh(Vh @ xT + A @ rhT + bh)
    outT = hT + eps * zT * dhT
so every matmul has the (pre-transposed, host-prepared) weight tile as the
stationary operand and xT/hT/rhT as the moving operand, and nothing ever
needs an on-device transpose.  The host transposes each core's [1024, 2048]
result back when assembling the full output.

Precision: the sigmoid-gate GEMMs (z, r) and the A@(r*h) term run in fp8
e4m3 with DoubleRow perf mode (2 contraction rows/cycle); inputs are
pre-scaled (x*16, W*256) and the scale is folded back in the activation
(scale=1/4096), which also applies the per-partition bias.  V_h@x (whose
error tanh passes through undamped) is split: half its contraction chunks
run in fp8 DoubleRow inside the A-path PSUM group, half in bf16 in a
separate group (VH_FP8 tunes the speed/accuracy dial).  All accumulation
is fp32 in PSUM; the residual h + eps*z*dh is fp32 on the vector engine.
Measured rel err vs the fp32 reference: 1.26e-2 (gate: 2e-2).
"""

import sys

sys.path.insert(0, "/opt/trn_rl_repo")

import numpy as np
import ml_dtypes

from contextlib import ExitStack

import concourse.bass as bass
import concourse.mybir as mybir
from concourse import bacc, tile
from concourse.bass import ds, ts
from concourse.bass_utils import run_bass_kernel_spmd

BF16 = mybir.dt.bfloat16
FP8 = mybir.dt.float8e4
F32 = mybir.dt.float32
AFT = mybir.ActivationFunctionType
ALU = mybir.AluOpType
DR = mybir.MatmulPerfMode.DoubleRow

# fp8 pre-scaling for the sigmoid-gate GEMMs (z, r): data*16, weights*256,
# compensated by activation scale 1/(16*256).
SCALE_X = 16.0
SCALE_W = 256.0
INV_SCALE = 1.0 / (SCALE_X * SCALE_W)

# V_h@x contraction chunks 0..VH_FP8-1 run in fp8 DoubleRow (folded into the
# A-path PSUM group, same mode/scale); chunks VH_FP8..7 run in bf16. fp8 dot
# error scales with sqrt(VH_FP8/8): 0 -> rel err 9.1e-3, 4 -> 1.26e-2
# (measured), 6 -> ~1.40e-2, 8 -> 1.53e-2 (measured); gate is 2e-2.
VH_FP8 = 6

N_CORES = 8
BATCH = 16384
B = BATCH // N_CORES  # per-core batch shard (2048)
H = 1024  # hidden == input size
KC = H // 128  # contraction chunks (8)
JT = H // 128  # output row tiles (8)
NB = 4  # moving-dim (batch) blocks per psum bank
NBS = B // NB  # 512 columns per matmul
GAMMA = 0.01

_nc_cache = {}


def _build(eps: float):
    """Build + compile the single-core Tile program (same graph on all cores)."""
    nc = bacc.Bacc("TRN2", target_bir_lowering=False, debug=False)

    xT_d = nc.dram_tensor("xT", [128, KC, B], BF16, kind="ExternalInput")
    hT_d = nc.dram_tensor("hT", [128, KC, B], BF16, kind="ExternalInput")
    # fp8 activations arrive batch-block-major so phase 1 can start on the
    # first 512-column block instead of waiting for the full stream.
    xT8_d = nc.dram_tensor("xT8", [NB, 128, KC, NBS], FP8, kind="ExternalInput")
    hT8_d = nc.dram_tensor("hT8", [NB, 128, KC, NBS], FP8, kind="ExternalInput")
    w_d = {
        name: nc.dram_tensor(name, [JT, 128, KC, 128], BF16, kind="ExternalInput")
        for name in ["vhT"]
    }
    w8_d = {
        name: nc.dram_tensor(name, [JT, 128, KC, 128], FP8, kind="ExternalInput")
        for name in ["wz8", "uz8", "wr8", "ur8", "at8", "vh8"]
    }
    bias_d = nc.dram_tensor("biases", [128, 24], F32, kind="ExternalInput")
    out_d = nc.dram_tensor("out", [H, B], F32, kind="ExternalOutput")

    with tile.TileContext(nc) as tc, ExitStack() as ctx:
        singles = ctx.enter_context(tc.tile_pool(name="singles", bufs=1))
        wpool = ctx.enter_context(tc.tile_pool(name="wpool", bufs=7))
        rwpool = ctx.enter_context(tc.tile_pool(name="rwpool", bufs=16))
        psum = ctx.enter_context(tc.tile_pool(name="psum", bufs=8, space="PSUM"))
        actp = ctx.enter_context(tc.tile_pool(name="actp", bufs=10))
        tmpp = ctx.enter_context(tc.tile_pool(name="tmpp", bufs=4))
        outp = ctx.enter_context(tc.tile_pool(name="outp", bufs=2))

        xT = singles.tile([128, KC, B], BF16)
        hTb = singles.tile([128, KC, B], BF16)
        xT8 = singles.tile([128, KC, B], FP8)
        hT8 = singles.tile([128, KC, B], FP8)
        rhT8 = singles.tile([128, KC, B], FP8)
        bias_sb = singles.tile([128, 24], F32)

        def load_w(name, jt):
            # gpsimd (SWDGE): keeps weight blocks off the Sync HWDGE queue,
            # which is busy issuing the bulk x/h stream.
            fp8 = name in w8_d
            w = wpool.tile([128, KC, 128], FP8 if fp8 else BF16, tag="w")
            nc.gpsimd.dma_start(out=w[:], in_=(w8_d[name] if fp8 else w_d[name])[jt])
            return w

        def load_rw(name, jt):
            # phase-1 weights stay resident across all 4 batch-block passes
            w = rwpool.tile([128, KC, 128], FP8, tag="rw")
            nc.sync.dma_start(out=w[:], in_=w8_d[name][jt])
            return w

        # Issue order on the sync queue matches consumption order: jt0's
        # r-weights, the first x8/h8 batch block, then the remaining
        # r-weights interleaved with the hTb chunks (pass nb0 consumes
        # rw[jt] and hTb[:,jt,:] in lockstep, one pair every ~3.4us).
        rw = [(load_rw("wr8", 0), load_rw("ur8", 0))]
        # first batch block split by contraction chunk-pairs, x before h,
        # matching the k-sweep consumption order of the first matmul group
        for kp in range(0, KC, 2):
            nc.sync.dma_start(
                out=xT8[:, kp : kp + 2, ds(0, NBS)], in_=xT8_d[0][:, kp : kp + 2, :]
            )
        for kp in range(0, KC, 2):
            nc.sync.dma_start(
                out=hT8[:, kp : kp + 2, ds(0, NBS)], in_=hT8_d[0][:, kp : kp + 2, :]
            )
        nc.sync.dma_start(out=bias_sb[:], in_=bias_d[:])
        nc.sync.dma_start(out=hTb[:, 0, :], in_=hT_d[:, 0, :])
        for jt in range(1, JT):
            rw.append((load_rw("wr8", jt), load_rw("ur8", jt)))
            nc.sync.dma_start(out=hTb[:, jt, :], in_=hT_d[:, jt, :])
        # remaining x8/h8 blocks (needed from pass nb1 on) and bf16 x (phase 2)
        for nb in range(1, NB):
            nc.sync.dma_start(out=xT8[:, :, ds(nb * NBS, NBS)], in_=xT8_d[nb])
            nc.sync.dma_start(out=hT8[:, :, ds(nb * NBS, NBS)], in_=hT8_d[nb])
        for c in range(VH_FP8, KC):
            nc.sync.dma_start(out=xT[:, c, :], in_=xT_d[:, c, :])

        def mm(psum_ap, w_ap, rhs_ap, start, stop, reload_w, perf_mode=None):
            # (LDWEIGHTS issue is left to legalization; measured to overlap
            # the matmul stream fully, so no manual dedupe/ordering needed.)
            return nc.tensor.matmul(
                psum_ap, w_ap, rhs_ap, start=start, stop=stop, perf_mode=perf_mode
            )

        def gemm_pair_fp8(psums, wA, rhsA, wB, rhsB):
            # fp8 DoubleRow, nb-outer: each PSUM bank completes (and is
            # evicted by the ACT) while later banks still accumulate.
            for nb in range(NB):
                for k in range(0, KC, 2):
                    mm(
                        psums[nb][:],
                        wA[:, k : k + 2, :],
                        rhsA[:, k : k + 2, ds(nb * NBS, NBS)],
                        start=(k == 0),
                        stop=False,
                        reload_w=False,
                        perf_mode=DR,
                    )
                for k in range(0, KC, 2):
                    mm(
                        psums[nb][:],
                        wB[:, k : k + 2, :],
                        rhsB[:, k : k + 2, ds(nb * NBS, NBS)],
                        start=False,
                        stop=(k == KC - 2),
                        reload_w=False,
                        perf_mode=DR,
                    )

        # ---- phase 1: r gate (hidden-major, fp8), rhT = sigmoid(...) * hT ----
        # Batch-block-outer, jt-inner: the first pass needs only the first
        # 512-column x8/h8 block plus the r weights, so the PE starts early
        # and never waits on the bulk DMA. The 8 jt groups of one pass
        # exactly fill the 8 PSUM banks.
        for nb in range(NB):
            for jt in range(JT):
                wr, ur = rw[jt]
                ps = psum.tile([128, NBS], F32, tag="ps", name=f"ps_r{jt}_{nb}")
                for k in range(0, KC, 2):
                    mm(
                        ps[:],
                        wr[:, k : k + 2, :],
                        xT8[:, k : k + 2, ds(nb * NBS, NBS)],
                        start=(k == 0),
                        stop=False,
                        reload_w=False,
                        perf_mode=DR,
                    )
                for k in range(0, KC, 2):
                    mm(
                        ps[:],
                        ur[:, k : k + 2, :],
                        hT8[:, k : k + 2, ds(nb * NBS, NBS)],
                        start=False,
                        stop=(k == KC - 2),
                        reload_w=False,
                        perf_mode=DR,
                    )
                rt = actp.tile([128, NBS], BF16, tag="act")
                nc.scalar.activation(
                    rt[:],
                    ps[:],
                    AFT.Sigmoid,
                    bias=bias_sb[:, 8 + jt : 9 + jt],
                    scale=INV_SCALE,
                )
                # rh in scaled fp8 for the DoubleRow A-matmul: (r*16)*h
                nc.vector.scalar_tensor_tensor(
                    rhT8[:, jt, ds(nb * NBS, NBS)],
                    rt[:],
                    SCALE_X,
                    hTb[:, jt, ds(nb * NBS, NBS)],
                    op0=ALU.mult,
                    op1=ALU.mult,
                )

        # ---- phase 2: z gate (fp8) + delta_h (bf16) + residual, per jt ----
        for jt in range(JT):
            wz = load_w("wz8", jt)
            uz = load_w("uz8", jt)
            vh = load_w("vhT", jt)
            vh8 = load_w("vh8", jt)
            at = load_w("at8", jt)
            psz = [
                psum.tile([128, NBS], F32, tag="ps", name=f"ps_z{jt}_{i}")
                for i in range(NB)
            ]
            gemm_pair_fp8(psz, wz, xT8, uz, hT8)
            # dh-pre: V_h@x chunks >= VH_FP8 in bf16 (V_h host-scaled by
            # 4096) in their own PSUM group (mixing perf modes in one
            # accumulation group faults the device); V_h@x chunks < VH_FP8
            # plus A@(r*h), all fp8 DoubleRow at the same 16*256 scale,
            # share the second group. DVE adds the two.
            psv = [
                psum.tile([128, NBS], F32, tag="ps", name=f"ps_v{jt}_{i}")
                for i in range(NB)
            ]
            vhs = []
            for nb in range(NB):
                for k in range(VH_FP8, KC):
                    mm(
                        psv[nb][:],
                        vh[:, k, :],
                        xT[:, k, ds(nb * NBS, NBS)],
                        start=(k == VH_FP8),
                        stop=(k == KC - 1),
                        reload_w=False,
                    )
                t = tmpp.tile([128, NBS], F32, tag="vhs", name=f"vhs{jt}_{nb}")
                nc.vector.tensor_copy(t[:], psv[nb][:])
                vhs.append(t)
            psd = [
                psum.tile([128, NBS], F32, tag="ps", name=f"ps_d{jt}_{i}")
                for i in range(NB)
            ]
            for nb in range(NB):
                for k in range(0, VH_FP8, 2):
                    mm(
                        psd[nb][:],
                        vh8[:, k : k + 2, :],
                        xT8[:, k : k + 2, ds(nb * NBS, NBS)],
                        start=(k == 0),
                        stop=False,
                        reload_w=False,
                        perf_mode=DR,
                    )
                for k in range(0, KC, 2):
                    mm(
                        psd[nb][:],
                        at[:, k : k + 2, :],
                        rhT8[:, k : k + 2, ds(nb * NBS, NBS)],
                        start=False,
                        stop=(k == KC - 2),
                        reload_w=False,
                        perf_mode=DR,
                    )
            ot = outp.tile([128, B], F32, tag="out")
            for nb in range(NB):
                zt = actp.tile([128, NBS], BF16, tag="act")
                nc.scalar.activation(
                    zt[:],
                    psz[nb][:],
                    AFT.Sigmoid,
                    bias=bias_sb[:, jt : jt + 1],
                    scale=INV_SCALE,
                )
                dsum = tmpp.tile([128, NBS], F32, tag="dsum")
                nc.vector.tensor_add(dsum[:], psd[nb][:], vhs[nb][:])
                dt_ = actp.tile([128, NBS], BF16, tag="act")
                nc.scalar.activation(
                    dt_[:],
                    dsum[:],
                    AFT.Tanh,
                    bias=bias_sb[:, 16 + jt : 17 + jt],
                    scale=INV_SCALE,
                )
                zdh = tmpp.tile([128, NBS], F32, tag="zdh")
                nc.vector.tensor_mul(zdh[:], zt[:], dt_[:])
                # out = (z*dh) * eps + h
                nc.vector.scalar_tensor_tensor(
                    ot[:, ds(nb * NBS, NBS)],
                    zdh[:],
                    float(eps),
                    hTb[:, jt, ds(nb * NBS, NBS)],
                    op0=ALU.mult,
                    op1=ALU.add,
                )
                nc.sync.dma_start(
                    out=out_d[ts(jt, 128), ds(nb * NBS, NBS)],
                    in_=ot[:, ds(nb * NBS, NBS)],
                )

    nc.compile()
    return nc


def _get_nc(eps: float):
    key = float(eps)
    if key not in _nc_cache:
        _nc_cache[key] = _build(key)
    return _nc_cache[key]


def _block_weight(wT, dtype, scale=1.0):
    # [1024, 1024] (contraction-major) -> [jt, p, c, j] st. blk[jt,p,c,j] = wT[c*128+p, jt*128+j]
    blk = wT.reshape(KC, 128, JT, 128).transpose(2, 1, 0, 3)
    if scale != 1.0:
        blk = blk * scale
    return np.ascontiguousarray(blk).astype(dtype)


def _block_data(m):
    # per-core [B, 1024] -> [p, c, b] st. blk[p,c,b] = m[b, c*128+p]
    return np.ascontiguousarray(m.T.reshape(KC, 128, B).transpose(1, 0, 2))


def _block_data_nb(m):
    # per-core [B, 1024] -> [nb, p, c, nbs] batch-block-major
    blk = m.T.reshape(KC, 128, NB, NBS).transpose(2, 1, 0, 3)
    return np.ascontiguousarray(blk)


def _prep_in_maps(x, h_prev, W_z, b_z, U_z, W_r, b_r, U_r, V_h, b_h, W_h):
    BF = ml_dtypes.bfloat16
    F8 = ml_dtypes.float8_e4m3
    x16 = np.asarray(x, np.float32).astype(BF)
    h16 = np.asarray(h_prev, np.float32).astype(BF)
    x8 = (np.asarray(x, np.float32) * SCALE_X).astype(F8)
    h8 = (np.asarray(h_prev, np.float32) * SCALE_X).astype(F8)

    A = W_h - W_h.T - GAMMA * np.eye(H, dtype=np.float32)
    shared = {
        "wz8": _block_weight(W_z.T, F8, SCALE_W),
        "uz8": _block_weight(U_z.T, F8, SCALE_W),
        "wr8": _block_weight(W_r.T, F8, SCALE_W),
        "ur8": _block_weight(U_r.T, F8, SCALE_W),
        "at8": _block_weight(A.T, F8, SCALE_W),
        "vh8": _block_weight(V_h.T, F8, SCALE_W),
        "vhT": _block_weight(V_h.T, BF, SCALE_X * SCALE_W),
        "biases": np.ascontiguousarray(
            np.concatenate(
                [
                    b_z.reshape(JT, 128).T,
                    b_r.reshape(JT, 128).T,
                    b_h.reshape(JT, 128).T,
                ],
                axis=1,
            ).astype(np.float32)
        ),
    }
    in_maps = []
    for c in range(N_CORES):
        sl = slice(c * B, (c + 1) * B)
        in_maps.append(
            {
                "xT": _block_data(x16[sl]),
                "hT": _block_data(h16[sl]),
                "xT8": _block_data_nb(x8[sl]),
                "hT8": _block_data_nb(h8[sl]),
                **shared,
            }
        )
    return in_maps


def run(inputs, trace=False):
    """Returns (full_output [16384,1024] f32, BassKernelResults)."""
    np_in = {k: np.asarray(v, np.float32) for k, v in inputs.items()}
    eps = float(np_in.pop("epsilon"))
    in_maps = _prep_in_maps(**np_in)
    nc = _get_nc(eps)
    res = run_bass_kernel_spmd(
        nc, in_maps, core_ids=list(range(N_CORES)), trace=trace
    )
    out = np.empty((BATCH, H), np.float32)
    for c in range(N_CORES):
        out[c * B : (c + 1) * B, :] = res.results[c]["out"].T
    return out, res


def kernel(**inputs) -> np.ndarray:
    out, _ = run(inputs, trace=False)
    return out
